# revision 69
# baseline (speedup 1.0000x reference)
"""DGCNN forward kernel for Trainium2 (8 NeuronCores, data-parallel over batch).

Each core processes one point cloud (N=2048 points) end to end:
  4x EdgeConv (KNN k=20 + 1x1 conv + BN + LeakyReLU(0.2) + max over k)
  -> concat -> 1x1 conv to 1024 + BN + LeakyReLU -> global max+mean pool
  -> MLP 2048-512-256-128-2 with LeakyReLU(0.01).

Key algebraic rewrite: for monotone BN (scale>0) and LeakyReLU,
  max_k f(W @ [nbr - ctr, ctr]) = lrelu(max_k(U'[idx_k]) + V' + t),
with U' = s*Wl @ x, V' = s*(Wr - Wl) @ x, s = g*rsqrt(v+eps), t = b - m*s.
This avoids materializing [N, K, 2C] edge features; only U' rows are
gathered (dma_gather from a DRAM table).

The gather (GpSimd SWDGE descriptor generation, ~8ns/row) is the hard
critical path: 64 gathers x 20.4us. Everything else is arranged to hide
under it: weight/BN prep for all layers is front-loaded, and each layer's
tile pipeline emits, as soon as a quad of 4 tiles has reduced, the y
activations for that quad plus the NEXT layer's pre-work (nsq, S-rhs,
U-table tiles) - so the next layer's first gather can start ~30us after
the previous layer's last one. conv5 is likewise interleaved into layer
3's pipeline per quad. The fp32 nsq distance bias rides the S matmul as
an extra contraction row (ones row in lhsT x nsq row in rhs), which
costs no PE cycles (cycles scale with rhs columns, not rows).
"""

import numpy as np
from contextlib import ExitStack

import concourse.bass as bass
import concourse.bacc as bacc
import concourse.tile as tile
from concourse import mybir
from concourse.bass_utils import run_bass_kernel_spmd
from concourse.masks import make_identity

F32 = mybir.dt.float32
BF16 = mybir.dt.bfloat16
FP16 = mybir.dt.float16
I16 = mybir.dt.int16
U32 = mybir.dt.uint32
AF = mybir.ActivationFunctionType
ALU = mybir.AluOpType
AX = mybir.AxisListType

B, N, KNN, P = 8, 2048, 20, 128
NT = N // P                      # 16 point tiles
NQ = 4                           # 512-col quads
EPS = 1e-5
NEG = -60000.0
CONV = [(64, 3), (64, 64), (128, 64), (256, 128)]   # (O, C) of edge convs
LIN = [(512, 2048), (256, 512), (128, 256), (2, 128)]
LRELU_CONV = 0.2
LRELU_HEAD = 0.01
# u-table dtype per layer: dma_gather needs elem_size_bytes % 256 == 0
UDT = [F32, F32, F32, FP16]


def _emit(nc, tc, t_in, t_w, t_out, dbg):
    with ExitStack() as ctx:
        const = ctx.enter_context(tc.tile_pool(name="const", bufs=1))
        pers = ctx.enter_context(tc.tile_pool(name="pers", bufs=1))
        mid = ctx.enter_context(ExitStack())   # closed after the layer loop
        prep = mid.enter_context(tc.tile_pool(name="prep", bufs=1))
        rot = mid.enter_context(tc.tile_pool(name="rot", bufs=2))

        ident = const.tile([P, P], F32)
        make_identity(nc, ident[:])
        ones_col = const.tile([P, 1], F32)
        nc.vector.memset(ones_col, 1.0)
        ones_row = const.tile([1, P], F32)
        nc.vector.memset(ones_row, 1.0)
        eps_col = const.tile([P, 1], F32)
        nc.vector.memset(eps_col, EPS)
        # SELR[g][p, p'] = 1 iff p == g*16 + p' % 16  (wrapped-idx builder)
        selr = const.tile([P, 8, P], F32)
        for g in range(8):
            isrc = ident[:, g * 16:(g + 1) * 16]
            src_b = bass.AP(tensor=isrc.tensor, offset=isrc.offset,
                            ap=[isrc.ap[0], [0, 8], isrc.ap[1]])
            nc.vector.tensor_copy(
                out=selr[:, g, :].rearrange("p (o q) -> p o q", q=16), in_=src_b)

        # persistent feature maps (channels-first: [C(part), N(free)])
        # fp32 master (feeds S/U/V matmuls) + fp16 copy (feeds conv5).
        # x_in/x0/x1 carry an all-ones row at partition 64: as S-matmul lhsT
        # it picks up the fp32 nsq row of the xr rhs, so the distance bias
        # rides the main matmul for free.
        x_cf = [
            pers.tile([65, N], F32, tag="x0", name="x0"),
            pers.tile([65, N], F32, tag="x1", name="x1"),
            pers.tile([P, N], F32, tag="x2", name="x2"),
            pers.tile([P, 2 * N], F32, tag="x3", name="x3"),  # 256 ch, 2 chunks
        ]
        x_bf = [
            pers.tile([64, N], FP16, tag="x0b", name="x0b"),
            pers.tile([64, N], FP16, tag="x1b", name="x1b"),
            pers.tile([P, N], FP16, tag="x2b", name="x2b"),
            pers.tile([P, 2 * N], FP16, tag="x3b", name="x3b"),
        ]
        x_in = pers.tile([65, N], F32, tag="x_in", name="x_in")
        nc.vector.memset(x_cf[0][64:65, :], 1.0)
        nc.vector.memset(x_cf[1][64:65, :], 1.0)
        nc.vector.memset(x_in[0:64, :], 0.0)
        nc.vector.memset(x_in[64:65, :], 1.0)
        # conv5 pooling partials: [p, j*16+t]
        pmax_parts = pers.tile([P, 8 * NT], F32, tag="pmax_parts")
        mean_parts = pers.tile([P, 8 * NT], F32, tag="mean_parts")
        p_cf = pers.tile([P, 16], F32, tag="p_cf")

        srcs_l = [x_in, x_cf[0], x_cf[1], x_cf[2]]
        xr_l = [None, None, None, None]    # S rhs (x rows + nsq row), li < 3
        # cross-layer S/topk pool: lets the NEXT layer's tile-0 S matrix and
        # topk be computed inside the previous layer's pipeline hooks, so the
        # boundary chain is reduce -> y(q3) -> S block 3 -> topk -> gather.
        sbS = mid.enter_context(tc.tile_pool(name="sbS", bufs=3))
        carry0 = {"i24": None}

        def emit_s_block(nli, t, q, s_sb, ps_pool):
            sl = slice(q * 512, (q + 1) * 512)
            pq = ps_pool.tile([P, 512], F32, tag="s_ps")
            if nli < 3:
                nc.tensor.matmul(out=pq,
                                 lhsT=srcs_l[nli][0:65, t * P:(t + 1) * P],
                                 rhs=xr_l[nli][0:65, sl],
                                 start=True, stop=True)
            else:
                nc.tensor.matmul(out=pq,
                                 lhsT=srcs_l[3][0:128, t * P:(t + 1) * P],
                                 rhs=srcs_l[3][0:128, sl],
                                 start=True, stop=False)
                nc.tensor.matmul(out=pq, lhsT=ones_row,
                                 rhs=xr_l[3][0:1, sl],
                                 start=False, stop=True)
            nc.scalar.activation(out=s_sb[:, sl], in_=pq, func=AF.Copy)

        def emit_topk(s_sb):
            v24 = sbS.tile([P, 24], F32, tag="v24", name="v24")
            i24 = sbS.tile([P, 24], U32, tag="i24", name="i24")
            nc.vector.max(out=v24[:, 0:8], in_=s_sb)
            nc.vector.max_index(out=i24[:, 0:8], in_max=v24[:, 0:8],
                                in_values=s_sb)
            nc.vector.match_replace(out=s_sb, in_to_replace=v24[:, 0:8],
                                    in_values=s_sb, imm_value=NEG)
            nc.vector.max(out=v24[:, 8:16], in_=s_sb)
            nc.vector.max_index(out=i24[:, 8:16], in_max=v24[:, 8:16],
                                in_values=s_sb)
            nc.vector.match_replace(out=s_sb, in_to_replace=v24[:, 8:16],
                                    in_values=s_sb, imm_value=NEG)
            nc.vector.max(out=v24[:, 16:24], in_=s_sb)
            nc.vector.max_index(out=i24[:, 16:24], in_max=v24[:, 16:24],
                                in_values=s_sb)
            return i24

        def transpose_to(ps_pool, tag, dst_ap, src_ap, rows_out):
            """dst[f, p] = src[p, f] via PE; src SBUF [p<=128, f<=128]."""
            pt = ps_pool.tile([P, P], F32, tag=tag)
            kdim = src_ap.shape[0]
            nc.tensor.transpose(out=pt[0:rows_out, 0:kdim], in_=src_ap,
                                identity=ident[0:kdim, 0:kdim])
            nc.scalar.activation(out=dst_ap, in_=pt[0:rows_out, 0:kdim], func=AF.Copy)

        # ========== weight/BN prep, deferred per layer ==========
        # Each prep_layer(li) closure is emitted inside the PREVIOUS layer's
        # pipeline (hidden under its gathers); only L0's runs at startup.
        chains = []   # (x_bf tile, rows, W4 col offset, free offset)
        wlT_l, wvT_l, bnt_l = [None] * 4, [None] * 4, [None] * 4
        w4T = []
        s4t4 = []

        def prep_layer(li, ps_pool):
            O, C = CONV[li]
            OCH = (O + P - 1) // P
            cols = {}
            for nm in "gbmv":
                cols[nm] = rot.tile([P, OCH], F32, tag=f"col_{nm}", name=f"col_{nm}")
                for j in range(OCH):
                    ow = min(P, O - j * P)
                    nc.sync.dma_start(out=cols[nm][0:ow, j:j + 1],
                                      in_=t_w[f"{nm}{li}"][j * P:j * P + ow, :])
            bn_s = prep.tile([P, OCH], F32, tag=f"bn_s{li}", name=f"bn_s{li}")
            bn_t = prep.tile([P, OCH], F32, tag=f"bn_t{li}", name=f"bn_t{li}")
            tmp = rot.tile([P, OCH], F32, tag="bn_tmp")
            nc.scalar.activation(out=tmp, in_=cols["v"], func=AF.Sqrt,
                                 bias=eps_col, scale=1.0)
            nc.vector.reciprocal(out=bn_s, in_=tmp)
            nc.vector.tensor_mul(bn_s, bn_s, cols["g"])
            nc.vector.tensor_mul(tmp, cols["m"], bn_s)
            nc.vector.tensor_sub(bn_t, cols["b"], tmp)
            wlT = prep.tile([P, O], F32, tag=f"wlT{li}", name=f"wlT{li}")
            wvT = prep.tile([P, O], F32, tag=f"wvT{li}", name=f"wvT{li}")
            for j in range(OCH):
                ow = min(P, O - j * P)
                wsb = rot.tile([P, 2 * C], F32, tag="w_in", bufs=1)
                nc.sync.dma_start(out=wsb[0:ow, :],
                                  in_=t_w[f"W{li}"][j * P:j * P + ow, :])
                wss = rot.tile([P, 2 * C], F32, tag="w_s", bufs=1)
                nc.scalar.activation(out=wss[0:ow, :], in_=wsb[0:ow, :],
                                     func=AF.Copy, scale=bn_s[0:ow, j:j + 1])
                transpose_to(ps_pool, "sm", wlT[0:C, j * P:j * P + ow],
                             wss[0:ow, 0:C], C)
                transpose_to(ps_pool, "sm", wvT[0:C, j * P:j * P + ow],
                             wss[0:ow, C:2 * C], C)
            nc.vector.tensor_sub(wvT[0:C, 0:O], wvT[0:C, 0:O], wlT[0:C, 0:O])
            wlT_l[li] = wlT
            wvT_l[li] = wvT
            bnt_l[li] = bn_t

        def prep_conv5(ps_pool):
            # BN fold + W4T per cat chain (rows scaled by s4)
            cols4 = {}
            for nm in "gbmv":
                cols4[nm] = rot.tile([P, 8], F32, tag=f"c4_{nm}", name=f"c4_{nm}")
                for j in range(8):
                    nc.sync.dma_start(out=cols4[nm][:, j:j + 1],
                                      in_=t_w[f"{nm}4"][j * P:(j + 1) * P, :])
            s4 = prep.tile([P, 8], F32, tag="s4", name="s4")
            t4 = prep.tile([P, 8], F32, tag="t4", name="t4")
            tmp4 = rot.tile([P, 8], F32, tag="bn_tmp4")
            nc.scalar.activation(out=tmp4, in_=cols4["v"], func=AF.Sqrt,
                                 bias=eps_col, scale=1.0)
            nc.vector.reciprocal(out=s4, in_=tmp4)
            nc.vector.tensor_mul(s4, s4, cols4["g"])
            nc.vector.tensor_mul(tmp4, cols4["m"], s4)
            nc.vector.tensor_sub(t4, cols4["b"], tmp4)
            chain_meta = [(0, 64, 0, 0), (1, 64, 64, 0), (2, 128, 128, 0),
                          (3, 128, 256, 0), (3, 128, 384, N)]
            w4T.extend(prep.tile([P, 1024], FP16, tag=f"w4T_{ci}",
                                 name=f"w4T_{ci}") for ci in range(5))
            chains.extend((x_bf[xi], crow, c0, fo)
                          for (xi, crow, c0, fo) in chain_meta)
            s4t4.extend([t4, s4])

        def prep_conv5_w(j2, ps_pool):
            # two W4 row-chunks' worth of scaled transposes (spread over
            # L1's quad hooks so no single boundary eats the scalar cost)
            chain_meta = [(0, 64, 0, 0), (1, 64, 64, 0), (2, 128, 128, 0),
                          (3, 128, 256, 0), (3, 128, 384, N)]
            for j in (2 * j2, 2 * j2 + 1):
                wsb4 = rot.tile([P, 512], F32, tag="w4_in", bufs=1)
                nc.sync.dma_start(out=wsb4, in_=t_w["W4"][j * P:(j + 1) * P, :])
                wss4 = rot.tile([P, 512], F32, tag="w4_s", bufs=1)
                nc.scalar.activation(out=wss4, in_=wsb4, func=AF.Copy,
                                     scale=s4t4[1][:, j:j + 1])
                for ci, (xi, crow, c0, fo) in enumerate(chain_meta):
                    transpose_to(ps_pool, "sm", w4T[ci][0:crow, j * P:(j + 1) * P],
                                 wss4[:, c0:c0 + crow], crow)

        # ========== cross-layer pre-work helpers ==========
        def emit_pre_block(li, q, ps_pool):
            """nsq (and xr rhs, li<3) for 512-col block q of layer li, from
            its input srcs_l[li]. For li<3, xr row 64 gets -0.5*colsum(x^2);
            rows 0:64 are the fp32 x rows (zeros padding for L0)."""
            src_l = srcs_l[li]
            C = CONV[li][1]
            sl = slice(q * 512, (q + 1) * 512)
            xx = rot.tile([P, 512], F32, tag="xx")
            nc.scalar.activation(out=xx[0:C, :], in_=src_l[0:C, sl], func=AF.Square)
            pq = ps_pool.tile([P, 512], F32, tag="sm")
            nc.tensor.matmul(out=pq[0:1, :], lhsT=ones_col[0:C, :], rhs=xx[0:C, :],
                             start=True, stop=True)
            if li < 3:
                xr = xr_l[li]
                nc.scalar.activation(out=xr[0:64, sl], in_=src_l[0:64, sl],
                                     func=AF.Copy)
                nc.scalar.activation(out=xr[64:65, sl], in_=pq[0:1, :],
                                     func=AF.Copy, scale=-0.5)
            else:
                nc.scalar.activation(out=xr_l[3][0:1, sl], in_=pq[0:1, :],
                                     func=AF.Copy, scale=-0.5)

        def emit_utab_tiles(li, tiles, ps_pool):
            """U' table rows for layer li -> DRAM (point-major [N, O])."""
            src_l = srcs_l[li]
            O, C = CONV[li]
            u_dram = t_w[f"Utab{li}"]
            for t in tiles:
                pu = ps_pool.tile([P, 512], F32, tag="sm")
                nc.tensor.matmul(out=pu[:, 0:O], lhsT=src_l[0:C, t * P:(t + 1) * P],
                                 rhs=wlT_l[li][0:C, 0:O], start=True, stop=True)
                usb = rot.tile([P, O], UDT[li], tag=f"usb{li}")
                nc.scalar.activation(out=usb, in_=pu[:, 0:O], func=AF.Copy)
                nc.sync.dma_start(out=u_dram[t * P:(t + 1) * P, :], in_=usb)

        def emit_conv5_tile(t, ps_c5):
            for j in range(8):
                ph = ps_c5.tile([P, P], F32, tag="h_ps")
                for ci, (xt, crow, c0, fo) in enumerate(chains):
                    nc.tensor.matmul(out=ph,
                                     lhsT=w4T[ci][0:crow, j * P:(j + 1) * P],
                                     rhs=xt[0:crow, fo + t * P:fo + (t + 1) * P],
                                     start=(ci == 0), stop=(ci == 4))
                hq = rot.tile([P, P], F32, tag="h_sb")
                cix = j * NT + t
                nc.scalar.activation(out=hq, in_=ph, func=AF.Prelu, scale=1.0,
                                     bias=s4t4[0][:, j:j + 1], alpha=LRELU_CONV,
                                     accum_out=mean_parts[:, cix:cix + 1])
                nc.vector.tensor_reduce(out=pmax_parts[:, cix:cix + 1], in_=hq,
                                        axis=AX.X, op=ALU.max)

        # ---------------- input transpose + L0 pre-work ----------------
        with tc.tile_pool(name="ps_setup", bufs=2, space="PSUM") as ps_setup, \
             tc.tile_pool(name="sb_setup", bufs=2) as sb_setup:
            ft_all = sb_setup.tile([P, NT, 3], F32, tag="feat")
            nc.sync.dma_start(
                out=ft_all[:, :, :],
                in_=t_in["feat_xyz"].rearrange("(t p) c -> p t c", p=P))
            prep_layer(0, ps_setup)
            for t in range(NT):
                transpose_to(ps_setup, "tr", x_in[0:3, t * P:(t + 1) * P],
                             ft_all[:, t, :], 3)
            xr_l[0] = rot.tile([65, N], F32, tag="xr32", name="xr32_0")
            for q in range(NQ):
                emit_pre_block(0, q, ps_setup)

        # =================== edge conv layers ===================
        for li, (O, C) in enumerate(CONV):
            OCH = (O + P - 1) // P
            udt = UDT[li]
            src = srcs_l[li]
            wvT, bn_t = wvT_l[li], bnt_l[li]
            u_dram = t_w[f"Utab{li}"]
            with ExitStack() as lctx:
                sb = lctx.enter_context(tc.tile_pool(name=f"sb_l{li}", bufs=1))
                sbw = lctx.enter_context(tc.tile_pool(name=f"sbw_l{li}", bufs=3))
                sbg = lctx.enter_context(tc.tile_pool(name=f"sbg_l{li}", bufs=3))
                ps_s = lctx.enter_context(tc.tile_pool(
                    name=f"ps_s{li}", bufs=3 if li < 3 else 2, space="PSUM"))
                ps_sm = lctx.enter_context(
                    tc.tile_pool(name=f"ps_sm{li}", bufs=2, space="PSUM"))
                ps_y = lctx.enter_context(
                    tc.tile_pool(name=f"ps_y{li}", bufs=2, space="PSUM"))
                ps_c5 = lctx.enter_context(tc.tile_pool(
                    name="ps_c5", bufs=2, space="PSUM")) if li == 3 else None

                m_all = sb.tile([P, NT * O], F32, tag="m_all")
                i24_t = [None] * NT
                gt_t = [None] * NT
                s0n = [None]
                dst = x_cf[li]
                dstb = x_bf[li]

                def stage_s_topk(t):
                    if t == 0 and carry0["i24"] is not None:
                        i24_t[0] = carry0["i24"]
                        carry0["i24"] = None
                        return
                    s_sb = sbS.tile([P, N], F32, tag="s_sb", name="s_sb")
                    for q in range(NQ):
                        emit_s_block(li, t, q, s_sb, ps_s)
                    i24_t[t] = emit_topk(s_sb)

                def stage_gather(t):
                    # idxf/w16 conversions run on DVE, not scalar: their only
                    # dependency is topk(t), so the in-order DVE stream never
                    # head-of-line blocks (the scalar stream would stall S
                    # copies behind them at layer boundaries).
                    idxf = sbw.tile([P, KNN], F32, tag="idxf")
                    nc.scalar.activation(out=idxf, in_=i24_t[t][:, 0:KNN],
                                         func=AF.Copy)
                    pw = ps_sm.tile([P, 8 * KNN], F32, tag="sm")
                    for g in range(8):
                        nc.tensor.matmul(
                            out=pw[:, :].rearrange("p (k g) -> p k g", g=8)[:, :, g],
                            lhsT=selr[:, g, :], rhs=idxf, start=True, stop=True,
                            skip_group_check=True)
                    w16 = sbw.tile([P, 8 * KNN], I16, tag="w16")
                    nc.scalar.activation(out=w16, in_=pw, func=AF.Copy)
                    gt = sbg.tile([P, KNN, O], udt, tag="gather")
                    nc.gpsimd.dma_gather(
                        out_ap=gt[:, :, :], in_ap=u_dram[:, :], idxs_ap=w16[:, :],
                        num_idxs=P * KNN, num_idxs_reg=P * KNN, elem_size=O,
                        single_packet=False)
                    gt_t[t] = gt

                def stage_reduce(t):
                    # contiguous max tree over k (strided tensor_reduce is ~4x
                    # slower than packed reads on DVE)
                    gt = gt_t[t]
                    gf = gt.rearrange("p k o -> p (k o)")
                    h1 = sbw.tile([P, 10 * O], udt, tag="red1")
                    nc.vector.tensor_tensor(out=h1, in0=gf[:, 0:10 * O],
                                            in1=gf[:, 10 * O:20 * O], op=ALU.max)
                    h2 = sbw.tile([P, 5 * O], udt, tag="red2")
                    nc.vector.tensor_tensor(out=h2, in0=h1[:, 0:5 * O],
                                            in1=h1[:, 5 * O:10 * O], op=ALU.max)
                    h3 = sbw.tile([P, 2 * O], udt, tag="red3")
                    nc.vector.tensor_tensor(out=h3, in0=h2[:, 0:2 * O],
                                            in1=h2[:, 2 * O:4 * O], op=ALU.max)
                    h4 = sbw.tile([P, O], udt, tag="red4")
                    nc.vector.tensor_tensor(out=h4, in0=h3[:, 0:O],
                                            in1=h3[:, O:2 * O], op=ALU.max)
                    nc.vector.tensor_tensor(out=m_all[:, t * O:(t + 1) * O],
                                            in0=h4, in1=h2[:, 4 * O:5 * O],
                                            op=ALU.max)

                def stage_y_quad(q):
                    for j in range(OCH):
                        ow = min(P, O - j * P)
                        py = ps_y.tile([P, 512], F32, tag="y_ps")
                        nc.tensor.matmul(out=py[0:ow, :],
                                         lhsT=wvT[0:C, j * P:j * P + ow],
                                         rhs=src[0:C, q * 512:(q + 1) * 512],
                                         start=True, stop=False)
                        for tt in range(4):
                            t = q * 4 + tt
                            msl = m_all[:, t * O + j * P: t * O + j * P + ow]
                            nc.tensor.matmul(
                                out=py[0:ow, tt * P:(tt + 1) * P],
                                lhsT=msl, rhs=ident,
                                is_transpose=True, start=False, stop=(tt == 3),
                                skip_group_check=True)
                        osl = slice(j * N + q * 512, j * N + (q + 1) * 512)
                        nc.scalar.activation(out=dst[:, osl][0:ow, :],
                                             in_=py[0:ow, :],
                                             func=AF.Prelu, scale=1.0,
                                             bias=bn_t[0:ow, j:j + 1],
                                             alpha=LRELU_CONV)
                        nc.scalar.activation(out=dstb[:, osl][0:ow, :],
                                             in_=py[0:ow, :],
                                             func=AF.Prelu, scale=1.0,
                                             bias=bn_t[0:ow, j:j + 1],
                                             alpha=LRELU_CONV)

                def stage_y_tile(t):
                    for j in range(OCH):
                        ow = min(P, O - j * P)
                        py = ps_y.tile([P, P], F32, tag="y_ps")
                        nc.tensor.matmul(out=py[0:ow, :],
                                         lhsT=wvT[0:C, j * P:j * P + ow],
                                         rhs=src[0:C, t * P:(t + 1) * P],
                                         start=True, stop=False)
                        msl = m_all[:, t * O + j * P: t * O + j * P + ow]
                        nc.tensor.matmul(out=py[0:ow, :], lhsT=msl, rhs=ident,
                                         is_transpose=True, start=False,
                                         stop=True, skip_group_check=True)
                        osl = slice(j * N + t * P, j * N + (t + 1) * P)
                        nc.scalar.activation(out=dst[:, osl][0:ow, :],
                                             in_=py[0:ow, :],
                                             func=AF.Prelu, scale=1.0,
                                             bias=bn_t[0:ow, j:j + 1],
                                             alpha=LRELU_CONV)
                        nc.scalar.activation(out=dstb[:, osl][0:ow, :],
                                             in_=py[0:ow, :],
                                             func=AF.Prelu, scale=1.0,
                                             bias=bn_t[0:ow, j:j + 1],
                                             alpha=LRELU_CONV)

                for i in range(NT + 3):
                    if li == 0 and i == 1:
                        # L0's U table overlaps topk(0) instead of preceding it
                        emit_utab_tiles(0, range(NT), ps_sm)
                    if li < 3 and i == NT:
                        s0n[0] = sbS.tile([P, N], F32, tag="s_sb",
                                          name="s_sb0n")
                        for b in range(3):
                            emit_s_block(li + 1, 0, b, s0n[0], ps_s)
                    if i < NT:
                        stage_s_topk(i)
                    if 1 <= i <= NT:
                        stage_gather(i - 1)
                    if i >= 3:
                        stage_reduce(i - 3)
                        if li == 3:
                            stage_y_tile(i - 3)
                            emit_conv5_tile(i - 3, ps_c5)
                        elif (i - 3) % 4 == 3:
                            q = (i - 3) // 4
                            if li == 0:
                                # hide the later layers' weight prep under
                                # L0's gather chain
                                if q < 3:
                                    prep_layer(q + 1, ps_sm)
                                else:
                                    prep_conv5(ps_sm)
                            if li == 1:
                                prep_conv5_w(q, ps_sm)
                            stage_y_quad(q)
                            if li < 3:
                                if q == 0:
                                    xr_l[li + 1] = rot.tile([65, N], F32,
                                                            tag="xr32",
                                                            name="xr32_n")
                                emit_pre_block(li + 1, q, ps_sm)
                                # next layer's tile-0 S/topk, inside this
                                # layer's pipeline (blocks 0:3 were emitted
                                # at i == NT, after the last in-loop s_sb
                                # generation, to keep the buffer rotation's
                                # lookahead intact); before the utab copies
                                # so topk starts as early as possible
                                if q == 3:
                                    emit_s_block(li + 1, 0, 3, s0n[0], ps_s)
                                    carry0["i24"] = emit_topk(s0n[0])
                                emit_utab_tiles(li + 1, range(4 * q, 4 * q + 4),
                                                ps_sm)
                if dbg:
                    drows = min(O, P)
                    nc.sync.dma_start(out=t_out[f"dbg_x{li}"][:, :],
                                      in_=dst[0:drows, :])

        mid.close()

        # =================== pooling finish ===================
        with tc.tile_pool(name="sb_pool", bufs=2) as sbpl:
            for j in range(8):
                nc.vector.tensor_reduce(out=p_cf[:, j:j + 1],
                                        in_=pmax_parts[:, NT * j:NT * (j + 1)],
                                        axis=AX.X, op=ALU.max)
                nc.vector.tensor_reduce(out=p_cf[:, 8 + j:9 + j],
                                        in_=mean_parts[:, NT * j:NT * (j + 1)],
                                        axis=AX.X, op=ALU.add)
            nc.vector.tensor_scalar_mul(p_cf[:, 8:16], p_cf[:, 8:16], 1.0 / N)
            if dbg:
                nc.sync.dma_start(out=t_out["dbg_p"][:, :], in_=p_cf[:, :])

        # =================== MLP head (broadcast + DVE dot-products) ==========
        with ExitStack() as hctx:
            sb = hctx.enter_context(tc.tile_pool(name="sb_head", bufs=1))
            sbw = hctx.enter_context(tc.tile_pool(name="sbw_head", bufs=2))
            ps_hd = hctx.enter_context(tc.tile_pool(name="ps_hd", bufs=2, space="PSUM"))

            def lin(name, src_col, incols, w_dram, out_dim, alpha):
                """dst [128, ceil(out/128)] = lrelu(alpha)(W @ src).
                src_col [128, incols] column tile (in_dim = 128*incols)."""
                in_dim = P * incols
                och = (out_dim + P - 1) // P
                orows = min(P, out_dim)
                # prefetch all weight chunks before the bcast build
                wsbs = []
                for ot in range(och):
                    orw = min(P, out_dim - ot * P)
                    wsb = sbw.tile([P, in_dim], F32, tag=f"{name}_w{ot}",
                                   name=f"{name}_w{ot}", bufs=1)
                    nc.sync.dma_start(out=wsb[0:orw, :],
                                      in_=w_dram[ot * P:ot * P + orw, :])
                    wsbs.append(wsb)
                # broadcast src over partitions: bcast[p', c] = src[c]
                bcast = sb.tile([P, in_dim], F32, tag=f"{name}_bc")
                for j in range(incols):
                    pT = ps_hd.tile([1, P], F32, tag="hd_tr")
                    nc.tensor.transpose(out=pT, in_=src_col[:, j:j + 1],
                                        identity=ident)
                    rowj = sbw.tile([1, P], F32, tag="hd_row")
                    nc.scalar.activation(out=rowj, in_=pT, func=AF.Copy)
                    pb = ps_hd.tile([P, P], F32, tag="hd_bc")
                    nc.tensor.matmul(out=pb, lhsT=ones_row, rhs=rowj,
                                     start=True, stop=True)
                    nc.scalar.activation(out=bcast[:, j * P:(j + 1) * P], in_=pb,
                                         func=AF.Copy)
                dst = sb.tile([P, och], F32, tag=f"{name}_out")
                for ot in range(och):
                    orw = min(P, out_dim - ot * P)
                    wsb = wsbs[ot]
                    prod = sbw.tile([P, in_dim], F32, tag=f"{name}_prod")
                    nc.vector.tensor_mul(prod[0:orw, :], wsb[0:orw, :], bcast[0:orw, :])
                    nc.vector.tensor_reduce(out=dst[0:orw, ot:ot + 1],
                                            in_=prod[0:orw, :], axis=AX.X, op=ALU.add)
                if alpha is not None:
                    tmp = sbw.tile([P, och], F32, tag=f"{name}_tmp")
                    nc.vector.tensor_scalar_mul(tmp[0:orows, :], dst[0:orows, :], alpha)
                    nc.vector.tensor_tensor(out=dst[0:orows, :], in0=dst[0:orows, :],
                                            in1=tmp[0:orows, :], op=ALU.max)
                return dst

            y1 = lin("y1", p_cf, 16, t_w["L1"], 512, LRELU_HEAD)
            y2 = lin("y2", y1, 4, t_w["L2"], 256, LRELU_HEAD)
            y3 = lin("y3", y2, 2, t_w["L3"], 128, LRELU_HEAD)
            y4 = lin("y4", y3, 1, t_w["L4"], 2, None)
            osb = sb.tile([2, 1], F32, tag="out_sb")
            nc.vector.tensor_copy(out=osb, in_=y4[0:2, 0:1])
            nc.sync.dma_start(out=t_out["out"][:, :], in_=osb)


_PROG_CACHE = {}


def _build(dbg=False):
    key = ("v4", dbg)
    if key in _PROG_CACHE:
        return _PROG_CACHE[key]
    nc = bacc.Bacc("TRN2", target_bir_lowering=False, debug=False, num_devices=B)
    t_in = {"feat_xyz": nc.declare_dram_parameter("feat_xyz", [N, 3], F32, isOutput=False)}
    t_w = {}
    for li, (O, C) in enumerate(CONV + [(1024, 512)]):
        wshape = [O, 2 * C] if li < 4 else [O, C]
        t_w[f"W{li}"] = nc.declare_dram_parameter(f"W{li}", wshape, F32, isOutput=False)
        for nm in "gbmv":
            t_w[f"{nm}{li}"] = nc.declare_dram_parameter(f"{nm}{li}", [O, 1], F32,
                                                         isOutput=False)
    for j, (o, c) in enumerate(LIN):
        t_w[f"L{j+1}"] = nc.declare_dram_parameter(f"L{j+1}", [o, c], F32, isOutput=False)
    for li, (O, C) in enumerate(CONV):
        t_w[f"Utab{li}"] = nc.dram_tensor(f"Utab{li}", [N, O], UDT[li])
    t_out = {"out": nc.declare_dram_parameter("out", [2, 1], F32, isOutput=True)}
    if dbg:
        for li, (O, C) in enumerate(CONV):
            sh = [P, 2 * N] if O == 256 else [O, N]
            t_out[f"dbg_x{li}"] = nc.declare_dram_parameter(f"dbg_x{li}", sh, F32,
                                                            isOutput=True)
        t_out["dbg_p"] = nc.declare_dram_parameter("dbg_p", [P, 16], F32, isOutput=True)

    with tile.TileContext(nc) as tc:
        _emit(nc, tc, t_in, t_w, t_out, dbg)
    nc.compile()
    _PROG_CACHE[key] = nc
    return nc


def _make_in_maps(inputs):
    feat = np.ascontiguousarray(np.asarray(inputs["feat_xyz"], dtype=np.float32))
    common = {}
    for li in range(5):
        common[f"W{li}"] = np.ascontiguousarray(np.asarray(inputs[f"W{li}"], np.float32))
        for nm in "gbmv":
            common[f"{nm}{li}"] = np.ascontiguousarray(
                np.asarray(inputs[f"{nm}{li}"], np.float32).reshape(-1, 1))
    for j in range(1, 5):
        common[f"L{j}"] = np.ascontiguousarray(np.asarray(inputs[f"L{j}"], np.float32))
    return [dict(common, feat_xyz=np.ascontiguousarray(feat[b])) for b in range(B)]


def run(inputs, dbg=False, trace=False, **kw):
    nc = _build(dbg)
    in_maps = _make_in_maps(inputs)
    return run_bass_kernel_spmd(nc, in_maps, list(range(B)), trace=trace, **kw)


def kernel(**inputs):
    res = run(inputs).results
    out = np.stack([res[b]["out"][:, 0] for b in range(B)], axis=0)
    return out.astype(np.float32)


# revision 70
# speedup vs baseline: 1.0597x; 1.0597x over previous
"""DGCNN forward kernel for Trainium2 (8 NeuronCores, data-parallel over batch).

Each core processes one point cloud (N=2048 points) end to end:
  4x EdgeConv (KNN k=20 + 1x1 conv + BN + LeakyReLU(0.2) + max over k)
  -> concat -> 1x1 conv to 1024 + BN + LeakyReLU -> global max+mean pool
  -> MLP 2048-512-256-128-2 with LeakyReLU(0.01).

Key algebraic rewrite: for monotone BN (scale>0) and LeakyReLU,
  max_k f(W @ [nbr - ctr, ctr]) = lrelu(max_k(U'[idx_k]) + V' + t),
with U' = s*Wl @ x, V' = s*(Wr - Wl) @ x, s = g*rsqrt(v+eps), t = b - m*s.
This avoids materializing [N, K, 2C] edge features; only U' rows are
gathered (dma_gather from a DRAM table).

The gather (GpSimd SWDGE descriptor generation, ~8ns/row) is the hard
critical path: 64 gathers x 20.4us. Everything else is arranged to hide
under it: weight/BN prep for all layers is front-loaded, and each layer's
tile pipeline emits, as soon as a quad of 4 tiles has reduced, the y
activations for that quad plus the NEXT layer's pre-work (nsq, S-rhs,
U-table tiles) - so the next layer's first gather can start ~30us after
the previous layer's last one. conv5 is likewise interleaved into layer
3's pipeline per quad. The fp32 nsq distance bias rides the S matmul as
an extra contraction row (ones row in lhsT x nsq row in rhs), which
costs no PE cycles (cycles scale with rhs columns, not rows).
"""

import numpy as np
from contextlib import ExitStack

import concourse.bass as bass
import concourse.bacc as bacc
import concourse.tile as tile
from concourse import mybir
from concourse.bass_utils import run_bass_kernel_spmd
from concourse.masks import make_identity

F32 = mybir.dt.float32
BF16 = mybir.dt.bfloat16
FP16 = mybir.dt.float16
I16 = mybir.dt.int16
U32 = mybir.dt.uint32
AF = mybir.ActivationFunctionType
ALU = mybir.AluOpType
AX = mybir.AxisListType

B, N, KNN, P = 8, 2048, 20, 128
NT = N // P                      # 16 point tiles
NQ = 4                           # 512-col quads
EPS = 1e-5
NEG = -60000.0
CONV = [(64, 3), (64, 64), (128, 64), (256, 128)]   # (O, C) of edge convs
LIN = [(512, 2048), (256, 512), (128, 256), (2, 128)]
LRELU_CONV = 0.2
LRELU_HEAD = 0.01
# u-table dtype per layer: dma_gather needs elem_size_bytes % 256 == 0
UDT = [F32, F32, F32, FP16]


def _emit(nc, tc, t_in, t_w, t_out, dbg):
    with ExitStack() as ctx:
        const = ctx.enter_context(tc.tile_pool(name="const", bufs=1))
        pers = ctx.enter_context(tc.tile_pool(name="pers", bufs=1))
        mid = ctx.enter_context(ExitStack())   # closed after the layer loop
        prep = mid.enter_context(tc.tile_pool(name="prep", bufs=1))
        rot = mid.enter_context(tc.tile_pool(name="rot", bufs=2))

        ident = const.tile([P, P], F32)
        make_identity(nc, ident[:])
        ones_col = const.tile([P, 1], F32)
        nc.vector.memset(ones_col, 1.0)
        ones_row = const.tile([1, P], F32)
        nc.vector.memset(ones_row, 1.0)
        eps_col = const.tile([P, 1], F32)
        nc.vector.memset(eps_col, EPS)
        # SELR[g][p, p'] = 1 iff p == g*16 + p' % 16  (wrapped-idx builder)
        selr = const.tile([P, 8, P], F32)
        for g in range(8):
            isrc = ident[:, g * 16:(g + 1) * 16]
            src_b = bass.AP(tensor=isrc.tensor, offset=isrc.offset,
                            ap=[isrc.ap[0], [0, 8], isrc.ap[1]])
            nc.vector.tensor_copy(
                out=selr[:, g, :].rearrange("p (o q) -> p o q", q=16), in_=src_b)

        # persistent feature maps (channels-first: [C(part), N(free)])
        # fp32 master (feeds S/U/V matmuls) + fp16 copy (feeds conv5).
        # x_in/x0/x1 carry an all-ones row at partition 64: as S-matmul lhsT
        # it picks up the fp32 nsq row of the xr rhs, so the distance bias
        # rides the main matmul for free.
        x_cf = [
            pers.tile([65, N], F32, tag="x0", name="x0"),
            pers.tile([65, N], F32, tag="x1", name="x1"),
            pers.tile([P, N], F32, tag="x2", name="x2"),
            pers.tile([P, 2 * N], F32, tag="x3", name="x3"),  # 256 ch, 2 chunks
        ]
        x_bf = [
            pers.tile([64, N], FP16, tag="x0b", name="x0b"),
            pers.tile([64, N], FP16, tag="x1b", name="x1b"),
            pers.tile([P, N], FP16, tag="x2b", name="x2b"),
            pers.tile([P, 2 * N], FP16, tag="x3b", name="x3b"),
        ]
        x_in = pers.tile([65, N], F32, tag="x_in", name="x_in")
        nc.vector.memset(x_cf[0][64:65, :], 1.0)
        nc.vector.memset(x_cf[1][64:65, :], 1.0)
        nc.vector.memset(x_in[0:64, :], 0.0)
        nc.vector.memset(x_in[64:65, :], 1.0)
        # conv5 pooling partials: [p, j*4+q]
        pmax_parts = pers.tile([P, 32], F32, tag="pmax_parts")
        mean_parts = pers.tile([P, 32], F32, tag="mean_parts")
        p_cf = pers.tile([P, 16], F32, tag="p_cf")

        srcs_l = [x_in, x_cf[0], x_cf[1], x_cf[2]]
        xr_l = [None, None, None, None]    # S rhs (x rows + nsq row), li < 3
        # cross-layer S/topk pool: lets the NEXT layer's tile-0 S matrix and
        # topk be computed inside the previous layer's pipeline hooks, so the
        # boundary chain is reduce -> y(q3) -> S block 3 -> topk -> gather.
        sbS = mid.enter_context(tc.tile_pool(name="sbS", bufs=3))
        carry0 = {"i24": None}

        def emit_s_block(nli, t, q, s_sb, ps_pool):
            sl = slice(q * 512, (q + 1) * 512)
            pq = ps_pool.tile([P, 512], F32, tag="s_ps")
            if nli < 3:
                nc.tensor.matmul(out=pq,
                                 lhsT=srcs_l[nli][0:65, t * P:(t + 1) * P],
                                 rhs=xr_l[nli][0:65, sl],
                                 start=True, stop=True)
            else:
                nc.tensor.matmul(out=pq,
                                 lhsT=srcs_l[3][0:128, t * P:(t + 1) * P],
                                 rhs=srcs_l[3][0:128, sl],
                                 start=True, stop=False)
                nc.tensor.matmul(out=pq, lhsT=ones_row,
                                 rhs=xr_l[3][0:1, sl],
                                 start=False, stop=True)
            nc.scalar.activation(out=s_sb[:, sl], in_=pq, func=AF.Copy)

        def emit_topk(s_sb):
            v24 = sbS.tile([P, 24], F32, tag="v24", name="v24")
            i24 = sbS.tile([P, 24], U32, tag="i24", name="i24")
            nc.vector.max(out=v24[:, 0:8], in_=s_sb)
            nc.vector.max_index(out=i24[:, 0:8], in_max=v24[:, 0:8],
                                in_values=s_sb)
            nc.vector.match_replace(out=s_sb, in_to_replace=v24[:, 0:8],
                                    in_values=s_sb, imm_value=NEG)
            nc.vector.max(out=v24[:, 8:16], in_=s_sb)
            nc.vector.max_index(out=i24[:, 8:16], in_max=v24[:, 8:16],
                                in_values=s_sb)
            nc.vector.match_replace(out=s_sb, in_to_replace=v24[:, 8:16],
                                    in_values=s_sb, imm_value=NEG)
            nc.vector.max(out=v24[:, 16:24], in_=s_sb)
            nc.vector.max_index(out=i24[:, 16:24], in_max=v24[:, 16:24],
                                in_values=s_sb)
            return i24

        def transpose_to(ps_pool, tag, dst_ap, src_ap, rows_out):
            """dst[f, p] = src[p, f] via PE; src SBUF [p<=128, f<=128]."""
            pt = ps_pool.tile([P, P], F32, tag=tag)
            kdim = src_ap.shape[0]
            nc.tensor.transpose(out=pt[0:rows_out, 0:kdim], in_=src_ap,
                                identity=ident[0:kdim, 0:kdim])
            nc.scalar.activation(out=dst_ap, in_=pt[0:rows_out, 0:kdim], func=AF.Copy)

        # ========== weight/BN prep, deferred per layer ==========
        # Each prep_layer(li) closure is emitted inside the PREVIOUS layer's
        # pipeline (hidden under its gathers); only L0's runs at startup.
        chains = []   # (x_bf tile, rows, W4 col offset, free offset)
        wlT_l, wvT_l, bnt_l = [None] * 4, [None] * 4, [None] * 4
        w4T = []
        s4t4 = []

        def prep_layer(li, ps_pool):
            O, C = CONV[li]
            OCH = (O + P - 1) // P
            cols = {}
            for nm in "gbmv":
                cols[nm] = rot.tile([P, OCH], F32, tag=f"col_{nm}", name=f"col_{nm}")
                for j in range(OCH):
                    ow = min(P, O - j * P)
                    nc.sync.dma_start(out=cols[nm][0:ow, j:j + 1],
                                      in_=t_w[f"{nm}{li}"][j * P:j * P + ow, :])
            bn_s = prep.tile([P, OCH], F32, tag=f"bn_s{li}", name=f"bn_s{li}")
            bn_t = prep.tile([P, OCH], F32, tag=f"bn_t{li}", name=f"bn_t{li}")
            tmp = rot.tile([P, OCH], F32, tag="bn_tmp")
            nc.scalar.activation(out=tmp, in_=cols["v"], func=AF.Sqrt,
                                 bias=eps_col, scale=1.0)
            nc.vector.reciprocal(out=bn_s, in_=tmp)
            nc.vector.tensor_mul(bn_s, bn_s, cols["g"])
            nc.vector.tensor_mul(tmp, cols["m"], bn_s)
            nc.vector.tensor_sub(bn_t, cols["b"], tmp)
            wlT = prep.tile([P, O], F32, tag=f"wlT{li}", name=f"wlT{li}")
            wvT = prep.tile([P, O], F32, tag=f"wvT{li}", name=f"wvT{li}")
            for j in range(OCH):
                ow = min(P, O - j * P)
                wsb = rot.tile([P, 2 * C], F32, tag="w_in", bufs=1)
                nc.sync.dma_start(out=wsb[0:ow, :],
                                  in_=t_w[f"W{li}"][j * P:j * P + ow, :])
                wss = rot.tile([P, 2 * C], F32, tag="w_s", bufs=1)
                nc.scalar.activation(out=wss[0:ow, :], in_=wsb[0:ow, :],
                                     func=AF.Copy, scale=bn_s[0:ow, j:j + 1])
                transpose_to(ps_pool, "sm", wlT[0:C, j * P:j * P + ow],
                             wss[0:ow, 0:C], C)
                transpose_to(ps_pool, "sm", wvT[0:C, j * P:j * P + ow],
                             wss[0:ow, C:2 * C], C)
            nc.vector.tensor_sub(wvT[0:C, 0:O], wvT[0:C, 0:O], wlT[0:C, 0:O])
            wlT_l[li] = wlT
            wvT_l[li] = wvT
            bnt_l[li] = bn_t

        def prep_conv5(ps_pool):
            # BN fold + W4T per cat chain (rows scaled by s4)
            cols4 = {}
            for nm in "gbmv":
                cols4[nm] = rot.tile([P, 8], F32, tag=f"c4_{nm}", name=f"c4_{nm}")
                for j in range(8):
                    nc.sync.dma_start(out=cols4[nm][:, j:j + 1],
                                      in_=t_w[f"{nm}4"][j * P:(j + 1) * P, :])
            s4 = prep.tile([P, 8], F32, tag="s4", name="s4")
            t4 = prep.tile([P, 8], F32, tag="t4", name="t4")
            tmp4 = rot.tile([P, 8], F32, tag="bn_tmp4")
            nc.scalar.activation(out=tmp4, in_=cols4["v"], func=AF.Sqrt,
                                 bias=eps_col, scale=1.0)
            nc.vector.reciprocal(out=s4, in_=tmp4)
            nc.vector.tensor_mul(s4, s4, cols4["g"])
            nc.vector.tensor_mul(tmp4, cols4["m"], s4)
            nc.vector.tensor_sub(t4, cols4["b"], tmp4)
            chain_meta = [(0, 64, 0, 0), (1, 64, 64, 0), (2, 128, 128, 0),
                          (3, 128, 256, 0), (3, 128, 384, N)]
            w4T.extend(prep.tile([P, 1024], FP16, tag=f"w4T_{ci}",
                                 name=f"w4T_{ci}") for ci in range(5))
            chains.extend((x_bf[xi], crow, c0, fo)
                          for (xi, crow, c0, fo) in chain_meta)
            s4t4.extend([t4, s4])

        def prep_conv5_w(j2, ps_pool):
            # two W4 row-chunks' worth of scaled transposes (spread over
            # L1's quad hooks so no single boundary eats the scalar cost)
            chain_meta = [(0, 64, 0, 0), (1, 64, 64, 0), (2, 128, 128, 0),
                          (3, 128, 256, 0), (3, 128, 384, N)]
            for j in (2 * j2, 2 * j2 + 1):
                wsb4 = rot.tile([P, 512], F32, tag="w4_in", bufs=1)
                nc.sync.dma_start(out=wsb4, in_=t_w["W4"][j * P:(j + 1) * P, :])
                wss4 = rot.tile([P, 512], F32, tag="w4_s", bufs=1)
                nc.scalar.activation(out=wss4, in_=wsb4, func=AF.Copy,
                                     scale=s4t4[1][:, j:j + 1])
                for ci, (xi, crow, c0, fo) in enumerate(chain_meta):
                    transpose_to(ps_pool, "sm", w4T[ci][0:crow, j * P:(j + 1) * P],
                                 wss4[:, c0:c0 + crow], crow)

        # ========== cross-layer pre-work helpers ==========
        def emit_pre_block(li, q, ps_pool):
            """nsq (and xr rhs, li<3) for 512-col block q of layer li, from
            its input srcs_l[li]. For li<3, xr row 64 gets -0.5*colsum(x^2);
            rows 0:64 are the fp32 x rows (zeros padding for L0)."""
            src_l = srcs_l[li]
            C = CONV[li][1]
            sl = slice(q * 512, (q + 1) * 512)
            xx = rot.tile([P, 512], F32, tag="xx")
            nc.scalar.activation(out=xx[0:C, :], in_=src_l[0:C, sl], func=AF.Square)
            pq = ps_pool.tile([P, 512], F32, tag="sm")
            nc.tensor.matmul(out=pq[0:1, :], lhsT=ones_col[0:C, :], rhs=xx[0:C, :],
                             start=True, stop=True)
            if li < 3:
                xr = xr_l[li]
                nc.scalar.activation(out=xr[0:64, sl], in_=src_l[0:64, sl],
                                     func=AF.Copy)
                nc.scalar.activation(out=xr[64:65, sl], in_=pq[0:1, :],
                                     func=AF.Copy, scale=-0.5)
            else:
                nc.scalar.activation(out=xr_l[3][0:1, sl], in_=pq[0:1, :],
                                     func=AF.Copy, scale=-0.5)

        def emit_utab_tiles(li, tiles, ps_pool):
            """U' table rows for layer li -> DRAM (point-major [N, O])."""
            src_l = srcs_l[li]
            O, C = CONV[li]
            u_dram = t_w[f"Utab{li}"]
            for t in tiles:
                pu = ps_pool.tile([P, 512], F32, tag="sm")
                nc.tensor.matmul(out=pu[:, 0:O], lhsT=src_l[0:C, t * P:(t + 1) * P],
                                 rhs=wlT_l[li][0:C, 0:O], start=True, stop=True)
                usb = rot.tile([P, O], UDT[li], tag=f"usb{li}")
                nc.scalar.activation(out=usb, in_=pu[:, 0:O], func=AF.Copy)
                nc.sync.dma_start(out=u_dram[t * P:(t + 1) * P, :], in_=usb)

        def emit_conv5_pair(q, j2, ps_c5):
            for j in (2 * j2, 2 * j2 + 1):
                ph = ps_c5.tile([P, 512], F32, tag="h_ps")
                for ci, (xt, crow, c0, fo) in enumerate(chains):
                    nc.tensor.matmul(out=ph,
                                     lhsT=w4T[ci][0:crow, j * P:(j + 1) * P],
                                     rhs=xt[0:crow, fo + q * 512:fo + (q + 1) * 512],
                                     start=(ci == 0), stop=(ci == 4))
                hq = rot.tile([P, 512], F32, tag="h_sb")
                cix = j * 4 + q
                nc.scalar.activation(out=hq, in_=ph, func=AF.Prelu, scale=1.0,
                                     bias=s4t4[0][:, j:j + 1], alpha=LRELU_CONV,
                                     accum_out=mean_parts[:, cix:cix + 1])
                nc.vector.tensor_reduce(out=pmax_parts[:, cix:cix + 1], in_=hq,
                                        axis=AX.X, op=ALU.max)

        # ---------------- input transpose + L0 pre-work ----------------
        with tc.tile_pool(name="ps_setup", bufs=2, space="PSUM") as ps_setup, \
             tc.tile_pool(name="sb_setup", bufs=2) as sb_setup:
            ft_all = sb_setup.tile([P, NT, 3], F32, tag="feat")
            nc.sync.dma_start(
                out=ft_all[:, :, :],
                in_=t_in["feat_xyz"].rearrange("(t p) c -> p t c", p=P))
            prep_layer(0, ps_setup)
            for t in range(NT):
                transpose_to(ps_setup, "tr", x_in[0:3, t * P:(t + 1) * P],
                             ft_all[:, t, :], 3)
            xr_l[0] = rot.tile([65, N], F32, tag="xr32", name="xr32_0")
            for q in range(NQ):
                emit_pre_block(0, q, ps_setup)

        # =================== edge conv layers ===================
        for li, (O, C) in enumerate(CONV):
            OCH = (O + P - 1) // P
            udt = UDT[li]
            src = srcs_l[li]
            wvT, bn_t = wvT_l[li], bnt_l[li]
            u_dram = t_w[f"Utab{li}"]
            with ExitStack() as lctx:
                sb = lctx.enter_context(tc.tile_pool(name=f"sb_l{li}", bufs=1))
                sbw = lctx.enter_context(tc.tile_pool(name=f"sbw_l{li}", bufs=3))
                sbg = lctx.enter_context(tc.tile_pool(name=f"sbg_l{li}", bufs=3))
                ps_s = lctx.enter_context(tc.tile_pool(
                    name=f"ps_s{li}", bufs=3 if li < 3 else 2, space="PSUM"))
                ps_sm = lctx.enter_context(
                    tc.tile_pool(name=f"ps_sm{li}", bufs=2, space="PSUM"))
                ps_y = lctx.enter_context(
                    tc.tile_pool(name=f"ps_y{li}", bufs=2, space="PSUM"))
                ps_c5 = lctx.enter_context(tc.tile_pool(
                    name="ps_c5", bufs=2, space="PSUM")) if li == 3 else None

                m_all = sb.tile([P, NT * O], F32, tag="m_all")
                i24_t = [None] * NT
                gt_t = [None] * NT
                s0n = [None]
                dst = x_cf[li]
                dstb = x_bf[li]

                def stage_s_topk(t):
                    if t == 0 and carry0["i24"] is not None:
                        i24_t[0] = carry0["i24"]
                        carry0["i24"] = None
                        return
                    s_sb = sbS.tile([P, N], F32, tag="s_sb", name="s_sb")
                    for q in range(NQ):
                        emit_s_block(li, t, q, s_sb, ps_s)
                    i24_t[t] = emit_topk(s_sb)

                def stage_gather(t):
                    # idxf/w16 conversions run on DVE, not scalar: their only
                    # dependency is topk(t), so the in-order DVE stream never
                    # head-of-line blocks (the scalar stream would stall S
                    # copies behind them at layer boundaries).
                    idxf = sbw.tile([P, KNN], F32, tag="idxf")
                    nc.scalar.activation(out=idxf, in_=i24_t[t][:, 0:KNN],
                                         func=AF.Copy)
                    pw = ps_sm.tile([P, 8 * KNN], F32, tag="sm")
                    for g in range(8):
                        nc.tensor.matmul(
                            out=pw[:, :].rearrange("p (k g) -> p k g", g=8)[:, :, g],
                            lhsT=selr[:, g, :], rhs=idxf, start=True, stop=True,
                            skip_group_check=True)
                    w16 = sbw.tile([P, 8 * KNN], I16, tag="w16")
                    nc.scalar.activation(out=w16, in_=pw, func=AF.Copy)
                    gt = sbg.tile([P, KNN, O], udt, tag="gather")
                    nc.gpsimd.dma_gather(
                        out_ap=gt[:, :, :], in_ap=u_dram[:, :], idxs_ap=w16[:, :],
                        num_idxs=P * KNN, num_idxs_reg=P * KNN, elem_size=O,
                        single_packet=False)
                    gt_t[t] = gt

                def stage_reduce(t):
                    # contiguous max tree over k (strided tensor_reduce is ~4x
                    # slower than packed reads on DVE)
                    gt = gt_t[t]
                    gf = gt.rearrange("p k o -> p (k o)")
                    h1 = sbw.tile([P, 10 * O], udt, tag="red1")
                    nc.vector.tensor_tensor(out=h1, in0=gf[:, 0:10 * O],
                                            in1=gf[:, 10 * O:20 * O], op=ALU.max)
                    h2 = sbw.tile([P, 5 * O], udt, tag="red2")
                    nc.vector.tensor_tensor(out=h2, in0=h1[:, 0:5 * O],
                                            in1=h1[:, 5 * O:10 * O], op=ALU.max)
                    h3 = sbw.tile([P, 2 * O], udt, tag="red3")
                    nc.vector.tensor_tensor(out=h3, in0=h2[:, 0:2 * O],
                                            in1=h2[:, 2 * O:4 * O], op=ALU.max)
                    h4 = sbw.tile([P, O], udt, tag="red4")
                    nc.vector.tensor_tensor(out=h4, in0=h3[:, 0:O],
                                            in1=h3[:, O:2 * O], op=ALU.max)
                    nc.vector.tensor_tensor(out=m_all[:, t * O:(t + 1) * O],
                                            in0=h4, in1=h2[:, 4 * O:5 * O],
                                            op=ALU.max)

                def stage_y_quad(q):
                    for j in range(OCH):
                        ow = min(P, O - j * P)
                        py = ps_y.tile([P, 512], F32, tag="y_ps")
                        nc.tensor.matmul(out=py[0:ow, :],
                                         lhsT=wvT[0:C, j * P:j * P + ow],
                                         rhs=src[0:C, q * 512:(q + 1) * 512],
                                         start=True, stop=False)
                        for tt in range(4):
                            t = q * 4 + tt
                            msl = m_all[:, t * O + j * P: t * O + j * P + ow]
                            nc.tensor.matmul(
                                out=py[0:ow, tt * P:(tt + 1) * P],
                                lhsT=msl, rhs=ident,
                                is_transpose=True, start=False, stop=(tt == 3),
                                skip_group_check=True)
                        osl = slice(j * N + q * 512, j * N + (q + 1) * 512)
                        nc.scalar.activation(out=dst[:, osl][0:ow, :],
                                             in_=py[0:ow, :],
                                             func=AF.Prelu, scale=1.0,
                                             bias=bn_t[0:ow, j:j + 1],
                                             alpha=LRELU_CONV)
                        nc.scalar.activation(out=dstb[:, osl][0:ow, :],
                                             in_=py[0:ow, :],
                                             func=AF.Prelu, scale=1.0,
                                             bias=bn_t[0:ow, j:j + 1],
                                             alpha=LRELU_CONV)

                c5_pending = []
                for i in range(NT + 3):
                    if li == 0 and i == 1:
                        # L0's U table overlaps topk(0) instead of preceding it
                        emit_utab_tiles(0, range(NT), ps_sm)
                    if li < 3 and i == NT:
                        s0n[0] = sbS.tile([P, N], F32, tag="s_sb",
                                          name="s_sb0n")
                        for b in range(3):
                            emit_s_block(li + 1, 0, b, s0n[0], ps_s)
                    if i < NT:
                        stage_s_topk(i)
                    if 1 <= i <= NT:
                        stage_gather(i - 1)
                    if i >= 3:
                        stage_reduce(i - 3)
                        if (i - 3) % 4 == 3:
                            q = (i - 3) // 4
                            if li == 0:
                                # hide the later layers' weight prep under
                                # L0's gather chain
                                if q < 3:
                                    prep_layer(q + 1, ps_sm)
                                else:
                                    prep_conv5(ps_sm)
                            if li == 1:
                                prep_conv5_w(q, ps_sm)
                            stage_y_quad(q)
                            if li < 3:
                                if q == 0:
                                    xr_l[li + 1] = rot.tile([65, N], F32,
                                                            tag="xr32",
                                                            name="xr32_n")
                                emit_pre_block(li + 1, q, ps_sm)
                                # next layer's tile-0 S/topk, inside this
                                # layer's pipeline (blocks 0:3 were emitted
                                # at i == NT, after the last in-loop s_sb
                                # generation, to keep the buffer rotation's
                                # lookahead intact); before the utab copies
                                # so topk starts as early as possible
                                if q == 3:
                                    emit_s_block(li + 1, 0, 3, s0n[0], ps_s)
                                    carry0["i24"] = emit_topk(s0n[0])
                                emit_utab_tiles(li + 1, range(4 * q, 4 * q + 4),
                                                ps_sm)
                            else:
                                c5_pending.extend((q, j2) for j2 in range(4))
                    if li == 3:
                        for _ in range(2):
                            if c5_pending:
                                emit_conv5_pair(*c5_pending.pop(0), ps_c5)
                if li == 3:
                    while c5_pending:
                        emit_conv5_pair(*c5_pending.pop(0), ps_c5)
                if dbg:
                    drows = min(O, P)
                    nc.sync.dma_start(out=t_out[f"dbg_x{li}"][:, :],
                                      in_=dst[0:drows, :])

        mid.close()

        # =================== pooling finish ===================
        with tc.tile_pool(name="sb_pool", bufs=2) as sbpl:
            for j in range(8):
                nc.vector.tensor_reduce(out=p_cf[:, j:j + 1],
                                        in_=pmax_parts[:, 4 * j:4 * j + 4],
                                        axis=AX.X, op=ALU.max)
                nc.vector.tensor_reduce(out=p_cf[:, 8 + j:9 + j],
                                        in_=mean_parts[:, 4 * j:4 * j + 4],
                                        axis=AX.X, op=ALU.add)
            nc.vector.tensor_scalar_mul(p_cf[:, 8:16], p_cf[:, 8:16], 1.0 / N)
            if dbg:
                nc.sync.dma_start(out=t_out["dbg_p"][:, :], in_=p_cf[:, :])

        # =================== MLP head (broadcast + DVE dot-products) ==========
        with ExitStack() as hctx:
            sb = hctx.enter_context(tc.tile_pool(name="sb_head", bufs=1))
            sbw = hctx.enter_context(tc.tile_pool(name="sbw_head", bufs=2))
            ps_hd = hctx.enter_context(tc.tile_pool(name="ps_hd", bufs=2, space="PSUM"))

            def lin(name, src_col, incols, w_dram, out_dim, alpha):
                """dst [128, ceil(out/128)] = lrelu(alpha)(W @ src).
                src_col [128, incols] column tile (in_dim = 128*incols)."""
                in_dim = P * incols
                och = (out_dim + P - 1) // P
                orows = min(P, out_dim)
                # prefetch all weight chunks before the bcast build
                wsbs = []
                for ot in range(och):
                    orw = min(P, out_dim - ot * P)
                    wsb = sbw.tile([P, in_dim], F32, tag=f"{name}_w{ot}",
                                   name=f"{name}_w{ot}", bufs=1)
                    nc.sync.dma_start(out=wsb[0:orw, :],
                                      in_=w_dram[ot * P:ot * P + orw, :])
                    wsbs.append(wsb)
                # broadcast src over partitions: bcast[p', c] = src[c]
                bcast = sb.tile([P, in_dim], F32, tag=f"{name}_bc")
                for j in range(incols):
                    pT = ps_hd.tile([1, P], F32, tag="hd_tr")
                    nc.tensor.transpose(out=pT, in_=src_col[:, j:j + 1],
                                        identity=ident)
                    rowj = sbw.tile([1, P], F32, tag="hd_row")
                    nc.scalar.activation(out=rowj, in_=pT, func=AF.Copy)
                    pb = ps_hd.tile([P, P], F32, tag="hd_bc")
                    nc.tensor.matmul(out=pb, lhsT=ones_row, rhs=rowj,
                                     start=True, stop=True)
                    nc.scalar.activation(out=bcast[:, j * P:(j + 1) * P], in_=pb,
                                         func=AF.Copy)
                dst = sb.tile([P, och], F32, tag=f"{name}_out")
                for ot in range(och):
                    orw = min(P, out_dim - ot * P)
                    wsb = wsbs[ot]
                    prod = sbw.tile([P, in_dim], F32, tag=f"{name}_prod")
                    nc.vector.tensor_mul(prod[0:orw, :], wsb[0:orw, :], bcast[0:orw, :])
                    nc.vector.tensor_reduce(out=dst[0:orw, ot:ot + 1],
                                            in_=prod[0:orw, :], axis=AX.X, op=ALU.add)
                if alpha is not None:
                    tmp = sbw.tile([P, och], F32, tag=f"{name}_tmp")
                    nc.vector.tensor_scalar_mul(tmp[0:orows, :], dst[0:orows, :], alpha)
                    nc.vector.tensor_tensor(out=dst[0:orows, :], in0=dst[0:orows, :],
                                            in1=tmp[0:orows, :], op=ALU.max)
                return dst

            y1 = lin("y1", p_cf, 16, t_w["L1"], 512, LRELU_HEAD)
            y2 = lin("y2", y1, 4, t_w["L2"], 256, LRELU_HEAD)
            y3 = lin("y3", y2, 2, t_w["L3"], 128, LRELU_HEAD)
            y4 = lin("y4", y3, 1, t_w["L4"], 2, None)
            osb = sb.tile([2, 1], F32, tag="out_sb")
            nc.vector.tensor_copy(out=osb, in_=y4[0:2, 0:1])
            nc.sync.dma_start(out=t_out["out"][:, :], in_=osb)


_PROG_CACHE = {}


def _build(dbg=False):
    key = ("v4", dbg)
    if key in _PROG_CACHE:
        return _PROG_CACHE[key]
    nc = bacc.Bacc("TRN2", target_bir_lowering=False, debug=False, num_devices=B)
    t_in = {"feat_xyz": nc.declare_dram_parameter("feat_xyz", [N, 3], F32, isOutput=False)}
    t_w = {}
    for li, (O, C) in enumerate(CONV + [(1024, 512)]):
        wshape = [O, 2 * C] if li < 4 else [O, C]
        t_w[f"W{li}"] = nc.declare_dram_parameter(f"W{li}", wshape, F32, isOutput=False)
        for nm in "gbmv":
            t_w[f"{nm}{li}"] = nc.declare_dram_parameter(f"{nm}{li}", [O, 1], F32,
                                                         isOutput=False)
    for j, (o, c) in enumerate(LIN):
        t_w[f"L{j+1}"] = nc.declare_dram_parameter(f"L{j+1}", [o, c], F32, isOutput=False)
    for li, (O, C) in enumerate(CONV):
        t_w[f"Utab{li}"] = nc.dram_tensor(f"Utab{li}", [N, O], UDT[li])
    t_out = {"out": nc.declare_dram_parameter("out", [2, 1], F32, isOutput=True)}
    if dbg:
        for li, (O, C) in enumerate(CONV):
            sh = [P, 2 * N] if O == 256 else [O, N]
            t_out[f"dbg_x{li}"] = nc.declare_dram_parameter(f"dbg_x{li}", sh, F32,
                                                            isOutput=True)
        t_out["dbg_p"] = nc.declare_dram_parameter("dbg_p", [P, 16], F32, isOutput=True)

    with tile.TileContext(nc) as tc:
        _emit(nc, tc, t_in, t_w, t_out, dbg)
    nc.compile()
    _PROG_CACHE[key] = nc
    return nc


def _make_in_maps(inputs):
    feat = np.ascontiguousarray(np.asarray(inputs["feat_xyz"], dtype=np.float32))
    common = {}
    for li in range(5):
        common[f"W{li}"] = np.ascontiguousarray(np.asarray(inputs[f"W{li}"], np.float32))
        for nm in "gbmv":
            common[f"{nm}{li}"] = np.ascontiguousarray(
                np.asarray(inputs[f"{nm}{li}"], np.float32).reshape(-1, 1))
    for j in range(1, 5):
        common[f"L{j}"] = np.ascontiguousarray(np.asarray(inputs[f"L{j}"], np.float32))
    return [dict(common, feat_xyz=np.ascontiguousarray(feat[b])) for b in range(B)]


def run(inputs, dbg=False, trace=False, **kw):
    nc = _build(dbg)
    in_maps = _make_in_maps(inputs)
    return run_bass_kernel_spmd(nc, in_maps, list(range(B)), trace=trace, **kw)


def kernel(**inputs):
    res = run(inputs).results
    out = np.stack([res[b]["out"][:, 0] for b in range(B)], axis=0)
    return out.astype(np.float32)


# revision 73
# speedup vs baseline: 1.0622x; 1.0023x over previous
"""DGCNN forward kernel for Trainium2 (8 NeuronCores, data-parallel over batch).

Each core processes one point cloud (N=2048 points) end to end:
  4x EdgeConv (KNN k=20 + 1x1 conv + BN + LeakyReLU(0.2) + max over k)
  -> concat -> 1x1 conv to 1024 + BN + LeakyReLU -> global max+mean pool
  -> MLP 2048-512-256-128-2 with LeakyReLU(0.01).

Key algebraic rewrite: for monotone BN (scale>0) and LeakyReLU,
  max_k f(W @ [nbr - ctr, ctr]) = lrelu(max_k(U'[idx_k]) + V' + t),
with U' = s*Wl @ x, V' = s*(Wr - Wl) @ x, s = g*rsqrt(v+eps), t = b - m*s.
This avoids materializing [N, K, 2C] edge features; only U' rows are
gathered (dma_gather from a DRAM table).

The gather (GpSimd SWDGE descriptor generation, ~8ns/row) is the hard
critical path: 64 gathers x 20.4us. Everything else is arranged to hide
under it: weight/BN prep for all layers is front-loaded, and each layer's
tile pipeline emits, as soon as a quad of 4 tiles has reduced, the y
activations for that quad plus the NEXT layer's pre-work (nsq, S-rhs,
U-table tiles) - so the next layer's first gather can start ~30us after
the previous layer's last one. conv5 is likewise interleaved into layer
3's pipeline per quad. The fp32 nsq distance bias rides the S matmul as
an extra contraction row (ones row in lhsT x nsq row in rhs), which
costs no PE cycles (cycles scale with rhs columns, not rows).
"""

import numpy as np
from contextlib import ExitStack

import concourse.bass as bass
import concourse.bacc as bacc
import concourse.tile as tile
from concourse import mybir
from concourse.bass_utils import run_bass_kernel_spmd
from concourse.masks import make_identity

F32 = mybir.dt.float32
BF16 = mybir.dt.bfloat16
FP16 = mybir.dt.float16
I16 = mybir.dt.int16
U32 = mybir.dt.uint32
AF = mybir.ActivationFunctionType
ALU = mybir.AluOpType
AX = mybir.AxisListType

B, N, KNN, P = 8, 2048, 20, 128
NT = N // P                      # 16 point tiles
NQ = 4                           # 512-col quads
EPS = 1e-5
NEG = -60000.0
CONV = [(64, 3), (64, 64), (128, 64), (256, 128)]   # (O, C) of edge convs
LIN = [(512, 2048), (256, 512), (128, 256), (2, 128)]
LRELU_CONV = 0.2
LRELU_HEAD = 0.01
# u-table dtype per layer: dma_gather needs elem_size_bytes % 256 == 0
UDT = [F32, F32, F32, FP16]


def _emit(nc, tc, t_in, t_w, t_out, dbg):
    with ExitStack() as ctx:
        const = ctx.enter_context(tc.tile_pool(name="const", bufs=1))
        pers = ctx.enter_context(tc.tile_pool(name="pers", bufs=1))
        mid = ctx.enter_context(ExitStack())   # closed after the layer loop
        prep = mid.enter_context(tc.tile_pool(name="prep", bufs=1))
        rot = mid.enter_context(tc.tile_pool(name="rot", bufs=2))

        ident = const.tile([P, P], F32)
        make_identity(nc, ident[:])
        ones_col = const.tile([P, 1], F32)
        nc.vector.memset(ones_col, 1.0)
        ones_row = const.tile([1, P], F32)
        nc.vector.memset(ones_row, 1.0)
        eps_col = const.tile([P, 1], F32)
        nc.vector.memset(eps_col, EPS)
        # SELR[g][p, p'] = 1 iff p == g*16 + p' % 16  (wrapped-idx builder)
        selr = const.tile([P, 8, P], F32)
        for g in range(8):
            isrc = ident[:, g * 16:(g + 1) * 16]
            src_b = bass.AP(tensor=isrc.tensor, offset=isrc.offset,
                            ap=[isrc.ap[0], [0, 8], isrc.ap[1]])
            nc.vector.tensor_copy(
                out=selr[:, g, :].rearrange("p (o q) -> p o q", q=16), in_=src_b)

        # persistent feature maps (channels-first: [C(part), N(free)])
        # fp32 master (feeds S/U/V matmuls) + fp16 copy (feeds conv5).
        # x_in/x0/x1 carry an all-ones row at partition 64: as S-matmul lhsT
        # it picks up the fp32 nsq row of the xr rhs, so the distance bias
        # rides the main matmul for free.
        x_cf = [
            pers.tile([65, N], F32, tag="x0", name="x0"),
            pers.tile([65, N], F32, tag="x1", name="x1"),
            pers.tile([P, N], F32, tag="x2", name="x2"),
            pers.tile([P, 2 * N], F32, tag="x3", name="x3"),  # 256 ch, 2 chunks
        ]
        x_bf = [
            pers.tile([64, N], FP16, tag="x0b", name="x0b"),
            pers.tile([64, N], FP16, tag="x1b", name="x1b"),
            pers.tile([P, N], FP16, tag="x2b", name="x2b"),
            pers.tile([P, 2 * N], FP16, tag="x3b", name="x3b"),
        ]
        x_in = pers.tile([65, N], F32, tag="x_in", name="x_in")
        nc.vector.memset(x_cf[0][64:65, :], 1.0)
        nc.vector.memset(x_cf[1][64:65, :], 1.0)
        nc.vector.memset(x_in[0:64, :], 0.0)
        nc.vector.memset(x_in[64:65, :], 1.0)
        # conv5 pooling partials: [p, j*4+q]
        pmax_parts = pers.tile([P, 32], F32, tag="pmax_parts")
        mean_parts = pers.tile([P, 32], F32, tag="mean_parts")
        p_cf = pers.tile([P, 16], F32, tag="p_cf")

        srcs_l = [x_in, x_cf[0], x_cf[1], x_cf[2]]
        xr_l = [None, None, None, None]    # S rhs (x rows + nsq row), li < 3
        # cross-layer S/topk pool: lets the NEXT layer's tile-0 S matrix and
        # topk be computed inside the previous layer's pipeline hooks, so the
        # boundary chain is reduce -> y(q3) -> S block 3 -> topk -> gather.
        sbS = mid.enter_context(tc.tile_pool(name="sbS", bufs=3))
        carry0 = {"i24": None}

        def emit_s_block(nli, t, q, s_sb, ps_pool):
            sl = slice(q * 512, (q + 1) * 512)
            pq = ps_pool.tile([P, 512], F32, tag="s_ps")
            if nli < 3:
                nc.tensor.matmul(out=pq,
                                 lhsT=srcs_l[nli][0:65, t * P:(t + 1) * P],
                                 rhs=xr_l[nli][0:65, sl],
                                 start=True, stop=True)
            else:
                nc.tensor.matmul(out=pq,
                                 lhsT=srcs_l[3][0:128, t * P:(t + 1) * P],
                                 rhs=srcs_l[3][0:128, sl],
                                 start=True, stop=False)
                nc.tensor.matmul(out=pq, lhsT=ones_row,
                                 rhs=xr_l[3][0:1, sl],
                                 start=False, stop=True)
            nc.scalar.activation(out=s_sb[:, sl], in_=pq, func=AF.Copy)

        def emit_topk(s_sb):
            v24 = sbS.tile([P, 24], F32, tag="v24", name="v24")
            i24 = sbS.tile([P, 24], U32, tag="i24", name="i24")
            nc.vector.max(out=v24[:, 0:8], in_=s_sb)
            nc.vector.max_index(out=i24[:, 0:8], in_max=v24[:, 0:8],
                                in_values=s_sb)
            nc.vector.match_replace(out=s_sb, in_to_replace=v24[:, 0:8],
                                    in_values=s_sb, imm_value=NEG)
            nc.vector.max(out=v24[:, 8:16], in_=s_sb)
            nc.vector.max_index(out=i24[:, 8:16], in_max=v24[:, 8:16],
                                in_values=s_sb)
            nc.vector.match_replace(out=s_sb, in_to_replace=v24[:, 8:16],
                                    in_values=s_sb, imm_value=NEG)
            nc.vector.max(out=v24[:, 16:24], in_=s_sb)
            nc.vector.max_index(out=i24[:, 16:24], in_max=v24[:, 16:24],
                                in_values=s_sb)
            return i24

        def transpose_to(ps_pool, tag, dst_ap, src_ap, rows_out):
            """dst[f, p] = src[p, f] via PE; src SBUF [p<=128, f<=128]."""
            pt = ps_pool.tile([P, P], F32, tag=tag)
            kdim = src_ap.shape[0]
            nc.tensor.transpose(out=pt[0:rows_out, 0:kdim], in_=src_ap,
                                identity=ident[0:kdim, 0:kdim])
            nc.scalar.activation(out=dst_ap, in_=pt[0:rows_out, 0:kdim], func=AF.Copy)

        # ========== weight/BN prep, deferred per layer ==========
        # Each prep_layer(li) closure is emitted inside the PREVIOUS layer's
        # pipeline (hidden under its gathers); only L0's runs at startup.
        chains = []   # (x_bf tile, rows, W4 col offset, free offset)
        wlT_l, wvT_l, bnt_l = [None] * 4, [None] * 4, [None] * 4
        w4T = []
        s4t4 = []

        def prep_layer(li, ps_pool):
            O, C = CONV[li]
            OCH = (O + P - 1) // P
            cols = {}
            for nm in "gbmv":
                cols[nm] = rot.tile([P, OCH], F32, tag=f"col_{nm}", name=f"col_{nm}")
                for j in range(OCH):
                    ow = min(P, O - j * P)
                    nc.sync.dma_start(out=cols[nm][0:ow, j:j + 1],
                                      in_=t_w[f"{nm}{li}"][j * P:j * P + ow, :])
            bn_s = prep.tile([P, OCH], F32, tag=f"bn_s{li}", name=f"bn_s{li}")
            bn_t = prep.tile([P, OCH], F32, tag=f"bn_t{li}", name=f"bn_t{li}")
            tmp = rot.tile([P, OCH], F32, tag="bn_tmp")
            nc.scalar.activation(out=tmp, in_=cols["v"], func=AF.Sqrt,
                                 bias=eps_col, scale=1.0)
            nc.vector.reciprocal(out=bn_s, in_=tmp)
            nc.vector.tensor_mul(bn_s, bn_s, cols["g"])
            nc.vector.tensor_mul(tmp, cols["m"], bn_s)
            nc.vector.tensor_sub(bn_t, cols["b"], tmp)
            wlT = prep.tile([P, O], F32, tag=f"wlT{li}", name=f"wlT{li}")
            wvT = prep.tile([P, O], F32, tag=f"wvT{li}", name=f"wvT{li}")
            for j in range(OCH):
                ow = min(P, O - j * P)
                wsb = rot.tile([P, 2 * C], F32, tag="w_in", bufs=1)
                nc.sync.dma_start(out=wsb[0:ow, :],
                                  in_=t_w[f"W{li}"][j * P:j * P + ow, :])
                wss = rot.tile([P, 2 * C], F32, tag="w_s", bufs=1)
                nc.scalar.activation(out=wss[0:ow, :], in_=wsb[0:ow, :],
                                     func=AF.Copy, scale=bn_s[0:ow, j:j + 1])
                transpose_to(ps_pool, "sm", wlT[0:C, j * P:j * P + ow],
                             wss[0:ow, 0:C], C)
                transpose_to(ps_pool, "sm", wvT[0:C, j * P:j * P + ow],
                             wss[0:ow, C:2 * C], C)
            nc.vector.tensor_sub(wvT[0:C, 0:O], wvT[0:C, 0:O], wlT[0:C, 0:O])
            wlT_l[li] = wlT
            wvT_l[li] = wvT
            bnt_l[li] = bn_t

        def prep_conv5(ps_pool):
            # BN fold + W4T per cat chain (rows scaled by s4)
            cols4 = {}
            for nm in "gbmv":
                cols4[nm] = rot.tile([P, 8], F32, tag=f"c4_{nm}", name=f"c4_{nm}")
                for j in range(8):
                    nc.sync.dma_start(out=cols4[nm][:, j:j + 1],
                                      in_=t_w[f"{nm}4"][j * P:(j + 1) * P, :])
            s4 = prep.tile([P, 8], F32, tag="s4", name="s4")
            t4 = prep.tile([P, 8], F32, tag="t4", name="t4")
            tmp4 = rot.tile([P, 8], F32, tag="bn_tmp4")
            nc.scalar.activation(out=tmp4, in_=cols4["v"], func=AF.Sqrt,
                                 bias=eps_col, scale=1.0)
            nc.vector.reciprocal(out=s4, in_=tmp4)
            nc.vector.tensor_mul(s4, s4, cols4["g"])
            nc.vector.tensor_mul(tmp4, cols4["m"], s4)
            nc.vector.tensor_sub(t4, cols4["b"], tmp4)
            chain_meta = [(0, 64, 0, 0), (1, 64, 64, 0), (2, 128, 128, 0),
                          (3, 128, 256, 0), (3, 128, 384, N)]
            w4T.extend(prep.tile([P, 1024], FP16, tag=f"w4T_{ci}",
                                 name=f"w4T_{ci}") for ci in range(5))
            chains.extend((x_bf[xi], crow, c0, fo)
                          for (xi, crow, c0, fo) in chain_meta)
            s4t4.extend([t4, s4])

        def prep_conv5_w(j2, ps_pool):
            # two W4 row-chunks' worth of scaled transposes (spread over
            # L1's quad hooks so no single boundary eats the scalar cost)
            chain_meta = [(0, 64, 0, 0), (1, 64, 64, 0), (2, 128, 128, 0),
                          (3, 128, 256, 0), (3, 128, 384, N)]
            for j in (2 * j2, 2 * j2 + 1):
                wsb4 = rot.tile([P, 512], F32, tag="w4_in", bufs=1)
                nc.sync.dma_start(out=wsb4, in_=t_w["W4"][j * P:(j + 1) * P, :])
                wss4 = rot.tile([P, 512], F32, tag="w4_s", bufs=1)
                nc.scalar.activation(out=wss4, in_=wsb4, func=AF.Copy,
                                     scale=s4t4[1][:, j:j + 1])
                for ci, (xi, crow, c0, fo) in enumerate(chain_meta):
                    transpose_to(ps_pool, "sm", w4T[ci][0:crow, j * P:(j + 1) * P],
                                 wss4[:, c0:c0 + crow], crow)

        # ========== cross-layer pre-work helpers ==========
        def emit_pre_block(li, q, ps_pool):
            """nsq (and xr rhs, li<3) for 512-col block q of layer li, from
            its input srcs_l[li]. For li<3, xr row 64 gets -0.5*colsum(x^2);
            rows 0:64 are the fp32 x rows (zeros padding for L0)."""
            src_l = srcs_l[li]
            C = CONV[li][1]
            sl = slice(q * 512, (q + 1) * 512)
            xx = rot.tile([P, 512], F32, tag="xx")
            nc.scalar.activation(out=xx[0:C, :], in_=src_l[0:C, sl], func=AF.Square)
            pq = ps_pool.tile([P, 512], F32, tag="sm")
            nc.tensor.matmul(out=pq[0:1, :], lhsT=ones_col[0:C, :], rhs=xx[0:C, :],
                             start=True, stop=True)
            if li < 3:
                xr = xr_l[li]
                nc.scalar.activation(out=xr[0:64, sl], in_=src_l[0:64, sl],
                                     func=AF.Copy)
                nc.scalar.activation(out=xr[64:65, sl], in_=pq[0:1, :],
                                     func=AF.Copy, scale=-0.5)
            else:
                nc.scalar.activation(out=xr_l[3][0:1, sl], in_=pq[0:1, :],
                                     func=AF.Copy, scale=-0.5)

        def emit_utab_tiles(li, tiles, ps_pool):
            """U' table rows for layer li -> DRAM (point-major [N, O])."""
            src_l = srcs_l[li]
            O, C = CONV[li]
            u_dram = t_w[f"Utab{li}"]
            for t in tiles:
                pu = ps_pool.tile([P, 512], F32, tag="sm")
                nc.tensor.matmul(out=pu[:, 0:O], lhsT=src_l[0:C, t * P:(t + 1) * P],
                                 rhs=wlT_l[li][0:C, 0:O], start=True, stop=True)
                usb = rot.tile([P, O], UDT[li], tag=f"usb{li}")
                nc.scalar.activation(out=usb, in_=pu[:, 0:O], func=AF.Copy)
                nc.sync.dma_start(out=u_dram[t * P:(t + 1) * P, :], in_=usb)

        def emit_conv5_pair(q, j2, ps_c5):
            for j in (2 * j2, 2 * j2 + 1):
                ph = ps_c5.tile([P, 512], F32, tag="h_ps")
                for ci, (xt, crow, c0, fo) in enumerate(chains):
                    nc.tensor.matmul(out=ph,
                                     lhsT=w4T[ci][0:crow, j * P:(j + 1) * P],
                                     rhs=xt[0:crow, fo + q * 512:fo + (q + 1) * 512],
                                     start=(ci == 0), stop=(ci == 4))
                hq = rot.tile([P, 512], F32, tag="h_sb")
                cix = j * 4 + q
                nc.scalar.activation(out=hq, in_=ph, func=AF.Prelu, scale=1.0,
                                     bias=s4t4[0][:, j:j + 1], alpha=LRELU_CONV,
                                     accum_out=mean_parts[:, cix:cix + 1])
                nc.vector.tensor_reduce(out=pmax_parts[:, cix:cix + 1], in_=hq,
                                        axis=AX.X, op=ALU.max)

        # ---------------- input transpose + L0 pre-work ----------------
        with tc.tile_pool(name="ps_setup", bufs=2, space="PSUM") as ps_setup, \
             tc.tile_pool(name="sb_setup", bufs=2) as sb_setup:
            ft_all = sb_setup.tile([P, NT, 3], F32, tag="feat")
            nc.sync.dma_start(
                out=ft_all[:, :, :],
                in_=t_in["feat_xyz"].rearrange("(t p) c -> p t c", p=P))
            prep_layer(0, ps_setup)
            for t in range(NT):
                transpose_to(ps_setup, "tr", x_in[0:3, t * P:(t + 1) * P],
                             ft_all[:, t, :], 3)
            xr_l[0] = rot.tile([65, N], F32, tag="xr32", name="xr32_0")
            for q in range(NQ):
                emit_pre_block(0, q, ps_setup)

        # =================== edge conv layers ===================
        for li, (O, C) in enumerate(CONV):
            OCH = (O + P - 1) // P
            udt = UDT[li]
            src = srcs_l[li]
            wvT, bn_t = wvT_l[li], bnt_l[li]
            u_dram = t_w[f"Utab{li}"]
            with ExitStack() as lctx:
                sb = lctx.enter_context(tc.tile_pool(name=f"sb_l{li}", bufs=1))
                sbw = lctx.enter_context(tc.tile_pool(name=f"sbw_l{li}", bufs=3))
                sbg = lctx.enter_context(tc.tile_pool(name=f"sbg_l{li}", bufs=3))
                ps_s = lctx.enter_context(tc.tile_pool(
                    name=f"ps_s{li}", bufs=3 if li < 3 else 2, space="PSUM"))
                ps_sm = lctx.enter_context(
                    tc.tile_pool(name=f"ps_sm{li}", bufs=2, space="PSUM"))
                ps_y = lctx.enter_context(
                    tc.tile_pool(name=f"ps_y{li}", bufs=2, space="PSUM"))
                ps_c5 = lctx.enter_context(tc.tile_pool(
                    name="ps_c5", bufs=2, space="PSUM")) if li == 3 else None

                m_all = sb.tile([P, NT * O], F32, tag="m_all")
                i24_t = [None] * NT
                gt_t = [None] * NT
                s0n = [None]
                dst = x_cf[li]
                dstb = x_bf[li]

                def stage_s_topk(t):
                    if t == 0 and carry0["i24"] is not None:
                        i24_t[0] = carry0["i24"]
                        carry0["i24"] = None
                        return
                    s_sb = sbS.tile([P, N], F32, tag="s_sb", name="s_sb")
                    for q in range(NQ):
                        emit_s_block(li, t, q, s_sb, ps_s)
                    i24_t[t] = emit_topk(s_sb)

                def stage_gather(t):
                    # idxf/w16 conversions run on DVE, not scalar: their only
                    # dependency is topk(t), so the in-order DVE stream never
                    # head-of-line blocks (the scalar stream would stall S
                    # copies behind them at layer boundaries).
                    idxf = sbw.tile([P, KNN], F32, tag="idxf")
                    nc.scalar.activation(out=idxf, in_=i24_t[t][:, 0:KNN],
                                         func=AF.Copy)
                    pw = ps_sm.tile([P, 8 * KNN], F32, tag="sm")
                    for g in range(8):
                        nc.tensor.matmul(
                            out=pw[:, :].rearrange("p (k g) -> p k g", g=8)[:, :, g],
                            lhsT=selr[:, g, :], rhs=idxf, start=True, stop=True,
                            skip_group_check=True)
                    w16 = sbw.tile([P, 8 * KNN], I16, tag="w16")
                    nc.scalar.activation(out=w16, in_=pw, func=AF.Copy)
                    gt = sbg.tile([P, KNN, O], udt, tag="gather")
                    nc.gpsimd.dma_gather(
                        out_ap=gt[:, :, :], in_ap=u_dram[:, :], idxs_ap=w16[:, :],
                        num_idxs=P * KNN, num_idxs_reg=P * KNN, elem_size=O,
                        single_packet=False)
                    gt_t[t] = gt

                def stage_reduce(t):
                    # contiguous max tree over k (strided tensor_reduce is ~4x
                    # slower than packed reads on DVE)
                    gt = gt_t[t]
                    gf = gt.rearrange("p k o -> p (k o)")
                    h1 = sbw.tile([P, 10 * O], udt, tag="red1")
                    nc.vector.tensor_tensor(out=h1, in0=gf[:, 0:10 * O],
                                            in1=gf[:, 10 * O:20 * O], op=ALU.max)
                    h2 = sbw.tile([P, 5 * O], udt, tag="red2")
                    nc.vector.tensor_tensor(out=h2, in0=h1[:, 0:5 * O],
                                            in1=h1[:, 5 * O:10 * O], op=ALU.max)
                    h3 = sbw.tile([P, 2 * O], udt, tag="red3")
                    nc.vector.tensor_tensor(out=h3, in0=h2[:, 0:2 * O],
                                            in1=h2[:, 2 * O:4 * O], op=ALU.max)
                    h4 = sbw.tile([P, O], udt, tag="red4")
                    nc.vector.tensor_tensor(out=h4, in0=h3[:, 0:O],
                                            in1=h3[:, O:2 * O], op=ALU.max)
                    nc.vector.tensor_tensor(out=m_all[:, t * O:(t + 1) * O],
                                            in0=h4, in1=h2[:, 4 * O:5 * O],
                                            op=ALU.max)

                def stage_y_quad(q):
                    for j in range(OCH):
                        ow = min(P, O - j * P)
                        py = ps_y.tile([P, 512], F32, tag="y_ps")
                        nc.tensor.matmul(out=py[0:ow, :],
                                         lhsT=wvT[0:C, j * P:j * P + ow],
                                         rhs=src[0:C, q * 512:(q + 1) * 512],
                                         start=True, stop=False)
                        for tt in range(4):
                            t = q * 4 + tt
                            msl = m_all[:, t * O + j * P: t * O + j * P + ow]
                            nc.tensor.matmul(
                                out=py[0:ow, tt * P:(tt + 1) * P],
                                lhsT=msl, rhs=ident,
                                is_transpose=True, start=False, stop=(tt == 3),
                                skip_group_check=True)
                        osl = slice(j * N + q * 512, j * N + (q + 1) * 512)
                        nc.scalar.activation(out=dst[:, osl][0:ow, :],
                                             in_=py[0:ow, :],
                                             func=AF.Prelu, scale=1.0,
                                             bias=bn_t[0:ow, j:j + 1],
                                             alpha=LRELU_CONV)
                        nc.scalar.activation(out=dstb[:, osl][0:ow, :],
                                             in_=py[0:ow, :],
                                             func=AF.Prelu, scale=1.0,
                                             bias=bn_t[0:ow, j:j + 1],
                                             alpha=LRELU_CONV)

                c5_pending = []
                for i in range(NT + 3):
                    if li == 0 and i == 1:
                        # L0's U table overlaps topk(0) instead of preceding it
                        emit_utab_tiles(0, range(NT), ps_sm)
                    if li < 3 and i == NT:
                        s0n[0] = sbS.tile([P, N], F32, tag="s_sb",
                                          name="s_sb0n")
                        for b in range(3):
                            emit_s_block(li + 1, 0, b, s0n[0], ps_s)
                    if i == 1:
                        # first gather's idx build ahead of topk(1) in the
                        # scalar/PE streams: at the boundary those engines
                        # are idle, so gather(0) launches ~15us earlier
                        stage_gather(0)
                    if i < NT:
                        stage_s_topk(i)
                    if 2 <= i <= NT:
                        stage_gather(i - 1)
                    if i >= 3:
                        stage_reduce(i - 3)
                        if (i - 3) % 4 == 3:
                            q = (i - 3) // 4
                            if li == 0:
                                # hide the later layers' weight prep under
                                # L0's gather chain
                                if q < 3:
                                    prep_layer(q + 1, ps_sm)
                                else:
                                    prep_conv5(ps_sm)
                            if li == 1:
                                prep_conv5_w(q, ps_sm)
                            stage_y_quad(q)
                            if li < 3:
                                if q == 0:
                                    xr_l[li + 1] = rot.tile([65, N], F32,
                                                            tag="xr32",
                                                            name="xr32_n")
                                emit_pre_block(li + 1, q, ps_sm)
                                # next layer's tile-0 S/topk, inside this
                                # layer's pipeline (blocks 0:3 were emitted
                                # at i == NT, after the last in-loop s_sb
                                # generation, to keep the buffer rotation's
                                # lookahead intact); before the utab copies
                                # so topk starts as early as possible
                                if q == 3:
                                    emit_s_block(li + 1, 0, 3, s0n[0], ps_s)
                                    carry0["i24"] = emit_topk(s0n[0])
                                emit_utab_tiles(li + 1, range(4 * q, 4 * q + 4),
                                                ps_sm)
                            else:
                                c5_pending.extend((q, j2) for j2 in range(4))
                    if li == 3:
                        for _ in range(2):
                            if c5_pending:
                                emit_conv5_pair(*c5_pending.pop(0), ps_c5)
                if li == 3:
                    while c5_pending:
                        emit_conv5_pair(*c5_pending.pop(0), ps_c5)
                if dbg:
                    drows = min(O, P)
                    nc.sync.dma_start(out=t_out[f"dbg_x{li}"][:, :],
                                      in_=dst[0:drows, :])

        mid.close()

        # =================== pooling finish ===================
        with tc.tile_pool(name="sb_pool", bufs=2) as sbpl:
            for j in range(8):
                nc.vector.tensor_reduce(out=p_cf[:, j:j + 1],
                                        in_=pmax_parts[:, 4 * j:4 * j + 4],
                                        axis=AX.X, op=ALU.max)
                nc.vector.tensor_reduce(out=p_cf[:, 8 + j:9 + j],
                                        in_=mean_parts[:, 4 * j:4 * j + 4],
                                        axis=AX.X, op=ALU.add)
            nc.vector.tensor_scalar_mul(p_cf[:, 8:16], p_cf[:, 8:16], 1.0 / N)
            if dbg:
                nc.sync.dma_start(out=t_out["dbg_p"][:, :], in_=p_cf[:, :])

        # =================== MLP head (broadcast + DVE dot-products) ==========
        with ExitStack() as hctx:
            sb = hctx.enter_context(tc.tile_pool(name="sb_head", bufs=1))
            sbw = hctx.enter_context(tc.tile_pool(name="sbw_head", bufs=2))
            ps_hd = hctx.enter_context(tc.tile_pool(name="ps_hd", bufs=2, space="PSUM"))

            def lin(name, src_col, incols, w_dram, out_dim, alpha):
                """dst [128, ceil(out/128)] = lrelu(alpha)(W @ src).
                src_col [128, incols] column tile (in_dim = 128*incols)."""
                in_dim = P * incols
                och = (out_dim + P - 1) // P
                orows = min(P, out_dim)
                # prefetch all weight chunks before the bcast build
                wsbs = []
                for ot in range(och):
                    orw = min(P, out_dim - ot * P)
                    wsb = sbw.tile([P, in_dim], F32, tag=f"{name}_w{ot}",
                                   name=f"{name}_w{ot}", bufs=1)
                    nc.sync.dma_start(out=wsb[0:orw, :],
                                      in_=w_dram[ot * P:ot * P + orw, :])
                    wsbs.append(wsb)
                # broadcast src over partitions: bcast[p', c] = src[c]
                bcast = sb.tile([P, in_dim], F32, tag=f"{name}_bc")
                for j in range(incols):
                    pT = ps_hd.tile([1, P], F32, tag="hd_tr")
                    nc.tensor.transpose(out=pT, in_=src_col[:, j:j + 1],
                                        identity=ident)
                    rowj = sbw.tile([1, P], F32, tag="hd_row")
                    nc.scalar.activation(out=rowj, in_=pT, func=AF.Copy)
                    pb = ps_hd.tile([P, P], F32, tag="hd_bc")
                    nc.tensor.matmul(out=pb, lhsT=ones_row, rhs=rowj,
                                     start=True, stop=True)
                    nc.scalar.activation(out=bcast[:, j * P:(j + 1) * P], in_=pb,
                                         func=AF.Copy)
                dst = sb.tile([P, och], F32, tag=f"{name}_out")
                for ot in range(och):
                    orw = min(P, out_dim - ot * P)
                    wsb = wsbs[ot]
                    prod = sbw.tile([P, in_dim], F32, tag=f"{name}_prod")
                    nc.vector.tensor_mul(prod[0:orw, :], wsb[0:orw, :], bcast[0:orw, :])
                    nc.vector.tensor_reduce(out=dst[0:orw, ot:ot + 1],
                                            in_=prod[0:orw, :], axis=AX.X, op=ALU.add)
                if alpha is not None:
                    tmp = sbw.tile([P, och], F32, tag=f"{name}_tmp")
                    nc.vector.tensor_scalar_mul(tmp[0:orows, :], dst[0:orows, :], alpha)
                    nc.vector.tensor_tensor(out=dst[0:orows, :], in0=dst[0:orows, :],
                                            in1=tmp[0:orows, :], op=ALU.max)
                return dst

            y1 = lin("y1", p_cf, 16, t_w["L1"], 512, LRELU_HEAD)
            y2 = lin("y2", y1, 4, t_w["L2"], 256, LRELU_HEAD)
            y3 = lin("y3", y2, 2, t_w["L3"], 128, LRELU_HEAD)
            y4 = lin("y4", y3, 1, t_w["L4"], 2, None)
            osb = sb.tile([2, 1], F32, tag="out_sb")
            nc.vector.tensor_copy(out=osb, in_=y4[0:2, 0:1])
            nc.sync.dma_start(out=t_out["out"][:, :], in_=osb)


_PROG_CACHE = {}


def _build(dbg=False):
    key = ("v4", dbg)
    if key in _PROG_CACHE:
        return _PROG_CACHE[key]
    nc = bacc.Bacc("TRN2", target_bir_lowering=False, debug=False, num_devices=B)
    t_in = {"feat_xyz": nc.declare_dram_parameter("feat_xyz", [N, 3], F32, isOutput=False)}
    t_w = {}
    for li, (O, C) in enumerate(CONV + [(1024, 512)]):
        wshape = [O, 2 * C] if li < 4 else [O, C]
        t_w[f"W{li}"] = nc.declare_dram_parameter(f"W{li}", wshape, F32, isOutput=False)
        for nm in "gbmv":
            t_w[f"{nm}{li}"] = nc.declare_dram_parameter(f"{nm}{li}", [O, 1], F32,
                                                         isOutput=False)
    for j, (o, c) in enumerate(LIN):
        t_w[f"L{j+1}"] = nc.declare_dram_parameter(f"L{j+1}", [o, c], F32, isOutput=False)
    for li, (O, C) in enumerate(CONV):
        t_w[f"Utab{li}"] = nc.dram_tensor(f"Utab{li}", [N, O], UDT[li])
    t_out = {"out": nc.declare_dram_parameter("out", [2, 1], F32, isOutput=True)}
    if dbg:
        for li, (O, C) in enumerate(CONV):
            sh = [P, 2 * N] if O == 256 else [O, N]
            t_out[f"dbg_x{li}"] = nc.declare_dram_parameter(f"dbg_x{li}", sh, F32,
                                                            isOutput=True)
        t_out["dbg_p"] = nc.declare_dram_parameter("dbg_p", [P, 16], F32, isOutput=True)

    with tile.TileContext(nc) as tc:
        _emit(nc, tc, t_in, t_w, t_out, dbg)
    nc.compile()
    _PROG_CACHE[key] = nc
    return nc


def _make_in_maps(inputs):
    feat = np.ascontiguousarray(np.asarray(inputs["feat_xyz"], dtype=np.float32))
    common = {}
    for li in range(5):
        common[f"W{li}"] = np.ascontiguousarray(np.asarray(inputs[f"W{li}"], np.float32))
        for nm in "gbmv":
            common[f"{nm}{li}"] = np.ascontiguousarray(
                np.asarray(inputs[f"{nm}{li}"], np.float32).reshape(-1, 1))
    for j in range(1, 5):
        common[f"L{j}"] = np.ascontiguousarray(np.asarray(inputs[f"L{j}"], np.float32))
    return [dict(common, feat_xyz=np.ascontiguousarray(feat[b])) for b in range(B)]


def run(inputs, dbg=False, trace=False, **kw):
    nc = _build(dbg)
    in_maps = _make_in_maps(inputs)
    return run_bass_kernel_spmd(nc, in_maps, list(range(B)), trace=trace, **kw)


def kernel(**inputs):
    res = run(inputs).results
    out = np.stack([res[b]["out"][:, 0] for b in range(B)], axis=0)
    return out.astype(np.float32)
